# revision 14
# baseline (speedup 1.0000x reference)
"""GQA attention kernel for Trainium2, data-parallel over batch on 8 NeuronCores.

Per-core problem (2 of 16 batches): X [1024tok, 1024] -> QKV proj -> RoPE ->
causal GQA attention (8 q heads, 4 kv heads, D=128) -> out proj [1024, 1024].

Layout strategy: everything stays in "feature-on-partition" transposed form,
and attention scores are computed TRANSPOSED (ST[tk,tq]) so that exp(ST) is
already the P.T the PV matmul needs -- no transposes of P at all. Matmul
operands are bf16 (fp32 PSUM accumulate).

Schedule (all per core):
  XT[hid,tok]   = host-pretransposed X                     (DRAM -> SBUF)
  QT[dq,tok]    = Wq.T @ XT   k-outer waves of 8 chains  + RoPE
  KT[dkv,tok]   = Wk.T @ XT   + RoPE
  V [tok,dkv]   = X @ Wv      (lhsT = XT, rhs = Wv)
  attention per (batch, kv-group) head PAIR, software-pipelined one pair deep:
    ST[tk, 2, tq] = KT_j.T @ QT_h  per 128-row tk block, causal col range,
                    both heads of the group into one 2-bank psum tile
    PT            = exp(ST)  one ACT op per (pair, j) via 3D AP
    mask          = one DVE mul per (pair, j) over both heads' diag blocks
                    (stride-0 broadcast mask operand)
    colsum[1,tq] += ones[128,1].T @ PT_j    (PE)
    OT[d,tq]     += V_j.T @ PT_j            (PE accumulate over j)
    norm per head, pipelined entirely off the PE:
      cs row -> scratch-DRAM roundtrip transpose -> WIDE reciprocal [128,4]
      -> roundtrip back -> DMA partition-broadcast [128,T] -> in-place
      SBUF multiply of the already-copied-out OT
  Out[tok,hid]  = OT.T @ Wo   -> bf16 store (host casts back to fp32)
RoPE scale 1/sqrt(D) is folded into the Q cos/sin host constants.
"""

import numpy as np
import ml_dtypes
from contextlib import ExitStack

import concourse.bass as bass
import concourse.tile as tile
from concourse import bacc, mybir
from concourse.bass_utils import run_bass_kernel_spmd

B, T, HID = 16, 512, 1024
NH, NKV, D = 8, 4, 128
THETA = 10000.0
NCORES = 8
BL = B // NCORES          # local batches per core
TOK = BL * T              # local tokens
P = 128
KT_HID = HID // P         # 8 contraction tiles over hidden
NTQ = T // P              # 4 tk/tq tiles per sequence
GROUPS = NH // NKV        # 2 q heads per kv head
NTOK_T = TOK // P         # 8 token tiles per core
FP32 = mybir.dt.float32
BF16 = mybir.dt.bfloat16
BF = ml_dtypes.bfloat16


def _host_consts():
    inv_freq = 1.0 / (THETA ** (np.arange(0, D, 2, dtype=np.float64) / D))
    freqs = np.outer(np.arange(T, dtype=np.float64), inv_freq)    # [T, 64]
    emb = np.concatenate([freqs, freqs], axis=-1)                 # [T, 128]
    cos = np.cos(emb).T                                           # [128, T]
    sin = np.sin(emb).T
    scale = 1.0 / np.sqrt(D)
    # rotate_half sign folded into sin: out = q*cos + qswap*sin_signed where
    # qswap is q with its partition halves swapped
    sin_signed = np.concatenate([-sin[:D // 2], sin[D // 2:]], axis=0)
    # transposed-S diagonal-block multiplicative mask: rows tk, cols tq;
    # valid iff tq >= tk
    mask_t = np.triu(np.ones((P, P), np.float32)).astype(BF)
    return {
        "cos_q": (cos * scale).astype(BF),
        "sin_q": (sin_signed * scale).astype(BF),
        "cos_k": cos.astype(BF),
        "sin_k": sin_signed.astype(BF),
        "mask_t": mask_t,
    }


def _rope(nc, out_sl, psum, cos_sb, sin_sb, tmp_pool):
    """out = q * cos + rotate_half(q) * sin for q = psum, all [128, T] APs.

    One ACT copy moves psum -> bf16 SBUF (single slow PSUM read), then the
    arithmetic runs in the DVE's fast bf16-SBUF mode.
    """
    H = D // 2
    qraw = tmp_pool.tile([P, T], BF16, tag="rope_raw")
    nc.scalar.copy(qraw, psum)
    # partition-half swap of rotate_half runs on the DMA engine (compute
    # engines cannot shift partitions between SBUF operands); the sign of
    # rotate_half is folded into the host sin constant
    qswap = tmp_pool.tile([P, T], BF16, tag="rope_swap")
    nc.sync.dma_start(out=qswap[0:H], in_=qraw[H:P])
    nc.sync.dma_start(out=qswap[H:P], in_=qraw[0:H])
    tmp = tmp_pool.tile([P, T], BF16, tag="rope_tmp")
    nc.gpsimd.tensor_mul(tmp, qswap, sin_sb)
    nc.vector.tensor_mul(out_sl, qraw, cos_sb)
    nc.vector.tensor_add(out_sl, out_sl, tmp)


def _build(nc):
    # hidden arrives pre-transposed from the host: [HID, TOK]
    hid_t = nc.dram_tensor("hidden_t", [HID, TOK], BF16,
                           kind="ExternalInput").ap()
    wq = nc.dram_tensor("Wq", [HID, NH * D], BF16, kind="ExternalInput").ap()
    wk = nc.dram_tensor("Wk", [HID, NKV * D], BF16, kind="ExternalInput").ap()
    wv = nc.dram_tensor("Wv", [HID, NKV * D], BF16, kind="ExternalInput").ap()
    wo = nc.dram_tensor("Wo", [NH * D, HID], BF16, kind="ExternalInput").ap()
    cos_q = nc.dram_tensor("cos_q", [P, T], BF16, kind="ExternalInput").ap()
    sin_q = nc.dram_tensor("sin_q", [P, T], BF16, kind="ExternalInput").ap()
    cos_k = nc.dram_tensor("cos_k", [P, T], BF16, kind="ExternalInput").ap()
    sin_k = nc.dram_tensor("sin_k", [P, T], BF16, kind="ExternalInput").ap()
    mask_t = nc.dram_tensor("mask_t", [P, P], BF16, kind="ExternalInput").ap()
    out = nc.dram_tensor("out", [TOK, HID], BF16, kind="ExternalOutput").ap()
    # scratch rows for the tiny row<->column transposes of the softmax
    # denominators (1KB per head; distinct rows avoid WAR between pipelined
    # head chains). All hops ride the sync DMA queue, which serializes them
    # without Tile needing to track the DRAM aliasing.
    cs_dram = nc.dram_tensor("cs_scratch", [BL * NH, T], BF16,
                             kind="Internal").ap()
    ri_dram = nc.dram_tensor("ri_scratch", [BL * NH, T], BF16,
                             kind="Internal").ap()

    with tile.TileContext(nc) as tc, ExitStack() as ctx:
        # ---- pools with cross-phase lifetimes ----
        consts = ctx.enter_context(tc.tile_pool(name="consts", bufs=1))

        cosq_sb = consts.tile([P, T], BF16, tag="cq")
        sinq_sb = consts.tile([P, T], BF16, tag="sq")
        cosk_sb = consts.tile([P, T], BF16, tag="ck")
        sink_sb = consts.tile([P, T], BF16, tag="sk")
        maskt_sb = consts.tile([P, P], BF16, tag="maskt")
        ones_bf = consts.tile([P, P], BF16, tag="ones")
        warm_rhs = consts.tile([P, 256], BF16, tag="warm")
        # memsets on the (otherwise idle at startup) gpsimd engine so the
        # PE warmup can begin as early as possible
        nc.gpsimd.memset(ones_bf, 1.0)
        nc.gpsimd.memset(warm_rhs, 0.0)

        qkvpool = ctx.enter_context(tc.tile_pool(name="qkv", bufs=1))
        qt_sb = qkvpool.tile([P, NH, TOK], BF16, tag="qt")     # [d, h, tok]
        kt_sb = qkvpool.tile([P, NKV, TOK], BF16, tag="kt")    # [d, g, tok]
        v_sb = qkvpool.tile([P, NTOK_T, NKV * D], BF16, tag="v")  # [tok,tt,dkv]

        # ---- phase A+B: loads + QKV projections (k-outer waves) ----
        with ExitStack() as phase1:
            wpool = phase1.enter_context(tc.tile_pool(name="wpool", bufs=1))
            xtp = phase1.enter_context(tc.tile_pool(name="xtp", bufs=1))
            ropet = phase1.enter_context(tc.tile_pool(name="ropet", bufs=4))
            psB = phase1.enter_context(
                tc.tile_pool(name="psB", bufs=8, space=bass.MemorySpace.PSUM))

            wq_sb = wpool.tile([P, KT_HID, NH * D], BF16, tag="wq")
            wk_sb = wpool.tile([P, KT_HID, NKV * D], BF16, tag="wk")
            wv_sb = wpool.tile([P, KT_HID, NKV * D], BF16, tag="wv")
            xt_sb = xtp.tile([P, KT_HID, TOK], BF16, tag="xt")  # [hid, k, tok]
            wq_r = wq.rearrange("(k p) n -> p k n", p=P)
            wk_r = wk.rearrange("(k p) n -> p k n", p=P)
            wv_r = wv.rearrange("(k p) n -> p k n", p=P)
            hid_r = hid_t.rearrange("(k p) t -> p k t", p=P)
            # load order follows consumption order: the Q projection streams
            # k-chunk by k-chunk, so (xt[k], wq[k]) pairs go first, split
            # across the two HARDWARE DGE rings (sync + scalar; the gpsimd
            # ring is software DGE executed on the Q7 cores -- never use it
            # for bulk). RoPE consts next; then wk/wv/wo.
            for k in range(KT_HID):
                nc.sync.dma_start(out=xt_sb[:, k, :], in_=hid_r[:, k, :])
                nc.scalar.dma_start(out=wq_sb[:, k, :], in_=wq_r[:, k, :])
            nc.sync.dma_start(out=cosq_sb, in_=cos_q)
            nc.sync.dma_start(out=sinq_sb, in_=sin_q)
            nc.scalar.dma_start(out=cosk_sb, in_=cos_k)
            nc.scalar.dma_start(out=sink_sb, in_=sin_k)
            nc.scalar.dma_start(out=maskt_sb, in_=mask_t)
            for k in range(KT_HID):
                nc.sync.dma_start(out=wk_sb[:, k, :], in_=wk_r[:, k, :])
                nc.scalar.dma_start(out=wv_sb[:, k, :], in_=wv_r[:, k, :])

            # PE warmup: ~3.5us of dependency-light matmuls ahead of the
            # first projection so the HAM clock-gate releases (1.2 ->
            # 2.4 GHz) while the input DMAs are still in flight
            wps = psB.tile([P, T], FP32, tag="projps")
            for _ in range(16):
                nc.tensor.matmul(wps[:, 0:256], ones_bf, warm_rhs,
                                 start=True, stop=True, skip_group_check=True)

            def q_wave(c):
                # k-outer: the first matmuls need only (xt[0], wq[0]), so
                # the PE starts when the first 512KB lands, not after the
                # whole load; DMA delivery and PE consumption are balanced.
                pss = [psB.tile([P, T], FP32, tag="projps",
                                name=f"psq{c}_{i}") for i in range(NH)]
                for k in range(KT_HID):
                    for h in range(NH):
                        nc.tensor.matmul(
                            pss[h],
                            wq_sb[:, k, h * P:(h + 1) * P],
                            xt_sb[:, k, c * T:(c + 1) * T],
                            start=(k == 0), stop=(k == KT_HID - 1))
                for h in range(NH):
                    _rope(nc, qt_sb[:, h, c * T:(c + 1) * T], pss[h],
                          cosq_sb, sinq_sb, ropet)

            def k_wave():
                chains = [(g, cc) for g in range(NKV) for cc in range(BL)]
                pss = [psB.tile([P, T], FP32, tag="projps", name=f"psk{i}")
                       for i in range(len(chains))]
                for k in range(KT_HID):
                    for i, (g, cc) in enumerate(chains):
                        nc.tensor.matmul(
                            pss[i],
                            wk_sb[:, k, g * P:(g + 1) * P],
                            xt_sb[:, k, cc * T:(cc + 1) * T],
                            start=(k == 0), stop=(k == KT_HID - 1))
                for i, (g, cc) in enumerate(chains):
                    _rope(nc, kt_sb[:, g, cc * T:(cc + 1) * T], pss[i],
                          cosk_sb, sink_sb, ropet)

            def v_wave():
                pss = [psB.tile([P, T], FP32, tag="projps", name=f"psv{i}")
                       for i in range(NTOK_T)]
                for k in range(KT_HID):
                    for tt in range(NTOK_T):
                        nc.tensor.matmul(
                            pss[tt][:, :NKV * D],
                            xt_sb[:, k, tt * P:(tt + 1) * P],
                            wv_sb[:, k, :],
                            start=(k == 0), stop=(k == KT_HID - 1))
                for tt in range(NTOK_T):
                    # alternate copy engines so the drain is not ACT-serial
                    if tt % 2 == 0:
                        nc.scalar.copy(v_sb[:, tt, :], pss[tt][:, :NKV * D])
                    else:
                        nc.vector.tensor_copy(v_sb[:, tt, :],
                                              pss[tt][:, :NKV * D])

            q_wave(0)
            k_wave()
            v_wave()
            q_wave(1)

        # ---- phase C: attention, head pairs, one-pair software pipeline ----
        otpool = ctx.enter_context(tc.tile_pool(name="otpool", bufs=1))
        ot_sb = otpool.tile([P, NH, TOK], BF16, tag="ot")      # [d, h, tok]
        wopool = ctx.enter_context(tc.tile_pool(name="wopool", bufs=1))
        wo_sb = wopool.tile([P, KT_HID, HID], BF16, tag="wo")
        nc.sync.dma_start(out=wo_sb, in_=wo.rearrange("(k p) n -> p k n", p=P))

        with ExitStack() as phase2:
            ptpool = phase2.enter_context(tc.tile_pool(name="ptpool", bufs=2))
            rowp = phase2.enter_context(tc.tile_pool(name="rowp", bufs=4))
            # PSUM budget is exactly 8 banks: ST pair tile 2 (bufs=1 is
            # enough -- by emission order the PE runs CSOT(p-1) between
            # ST(p) and ST(p+1), so exp(p) has long drained the tile) +
            # o_ps 2 + cs 2 + rank-1 broadcast 2.
            psS = phase2.enter_context(
                tc.tile_pool(name="psS", bufs=1, space=bass.MemorySpace.PSUM))
            psO = phase2.enter_context(
                tc.tile_pool(name="psO", bufs=2, space=bass.MemorySpace.PSUM))
            psC = phase2.enter_context(
                tc.tile_pool(name="psC", bufs=2, space=bass.MemorySpace.PSUM))
            psR = phase2.enter_context(
                tc.tile_pool(name="psR", bufs=2, space=bass.MemorySpace.PSUM))

            mask_b = maskt_sb[:, None, :].to_broadcast((P, GROUPS, P))

            def emit_st(b, g):
                """ST matmuls + exp + mask for one head pair; returns pt."""
                st = psS.tile([P, GROUPS, T], FP32, tag="sps")
                pt = ptpool.tile([P, GROUPS, NTQ, T], BF16, tag="pt")
                for j in range(NTQ):
                    lo = j * P
                    for hh in range(GROUPS):
                        h = GROUPS * g + hh
                        nc.tensor.matmul(
                            st[:, hh, lo:T],
                            kt_sb[:, g, b * T + lo: b * T + lo + P],
                            qt_sb[:, h, b * T + lo: (b + 1) * T],
                            start=True, stop=True)
                    # one exp per (pair, j): 3D AP spanning both psum banks
                    # (no row-max: logits are O(1) by construction)
                    nc.scalar.activation(
                        out=pt[:, :, j, lo:T], in_=st[:, :, lo:T],
                        func=mybir.ActivationFunctionType.Exp,
                        bias=0.0, scale=1.0)
                    # causal mask on the diagonal block, both heads in one
                    # op (mask operand broadcast along the head axis), on
                    # the otherwise-idle gpsimd
                    nc.gpsimd.tensor_mul(
                        pt[:, :, j, lo:lo + P], pt[:, :, j, lo:lo + P],
                        mask_b)
                return pt

            def emit_csot_mm(b, g, pt):
                """colsum + OT accumulation; denominator row shipped out.

                Returns per-head state for the deferred normalization."""
                states = []
                for hh in range(GROUPS):
                    h = GROUPS * g + hh
                    row = b * NH + h
                    o_ps = psO.tile([P, T], FP32, tag="ops")
                    cs_ps = psC.tile([1, T], FP32, tag="cps")
                    for j in range(NTQ):
                        lo = j * P
                        nc.tensor.matmul(
                            cs_ps[:, lo:T] if j else cs_ps[:, :],
                            ones_bf[:, 0:1],
                            pt[:, hh, j, lo:T],
                            start=(j == 0), stop=(j == NTQ - 1),
                            skip_group_check=True)
                        nc.tensor.matmul(
                            o_ps[:, lo:T] if j else o_ps[:, :],
                            v_sb[:, b * NTQ + j, g * D:(g + 1) * D],
                            pt[:, hh, j, lo:T],
                            start=(j == 0), stop=(j == NTQ - 1),
                            skip_group_check=True)
                    # unnormalized OT out of PSUM immediately (frees banks;
                    # the normalization multiply lands later, in-place)
                    nc.vector.tensor_copy(
                        ot_sb[:, h, b * T:(b + 1) * T], o_ps)
                    # denominator off to scratch DRAM: the [1,T] row is
                    # lane-starved on every compute engine (a [1,512]
                    # reciprocal costs ~3.4us!), so roundtrip through DRAM
                    # to transpose it into [128, 4] columns. Hops ride the
                    # sync queue, which orders the DRAM write->read.
                    csrow = rowp.tile([1, T], BF16, tag="csrow")
                    if hh == 0:
                        nc.scalar.copy(csrow, cs_ps)
                    else:
                        nc.vector.tensor_copy(csrow, cs_ps)
                    nc.sync.dma_start(out=cs_dram[row:row + 1, :], in_=csrow)
                    ccol = rowp.tile([P, NTQ], BF16, tag="ccol")
                    nc.sync.dma_start(
                        out=ccol,
                        in_=cs_dram[row].rearrange("(q p) -> p q", p=P))
                    states.append((h, row, ccol))
                return states

            def emit_norm(b, g, states):
                """wide reciprocal + broadcast + in-place multiply.

                Emitted one extra pair later than emit_csot_mm so that the
                in-order DVE/gpsimd queues never wait on the DMA roundtrip
                (the stall would block the next pair's mask/copy ops and
                idle the PE)."""
                for h, row, ccol in states:
                    rcol = rowp.tile([P, NTQ], BF16, tag="rcol")
                    with nc.allow_low_precision("bf16 softmax denominator"):
                        nc.vector.reciprocal(rcol, ccol)
                    nc.sync.dma_start(
                        out=ri_dram[row].rearrange("(q p) -> p q", p=P),
                        in_=rcol)
                    rrow = rowp.tile([1, T], BF16, tag="rrow")
                    nc.sync.dma_start(out=rrow,
                                      in_=ri_dram[row:row + 1, :])
                    # broadcast the reciprocal row to all partitions with a
                    # warm rank-1 matmul (~250ns; a DMA partition-broadcast
                    # of [128,T] costs ~8us in replicated descriptors!)
                    rb_ps = psR.tile([P, T], FP32, tag="rbps")
                    nc.tensor.matmul(rb_ps, ones_bf[0:1, :], rrow,
                                     start=True, stop=True,
                                     skip_group_check=True)
                    nc.vector.tensor_mul(
                        ot_sb[:, h, b * T:(b + 1) * T],
                        ot_sb[:, h, b * T:(b + 1) * T], rb_ps)

            pairs = [(b, g) for b in range(BL) for g in range(NKV)]
            hist = []  # [(b, g, pt or states), ...] pipeline registers
            for i, (b, g) in enumerate(pairs):
                pt = emit_st(b, g)
                if i >= 1:
                    pb, pg, ppt = hist[i - 1]
                    hist[i - 1] = (pb, pg, emit_csot_mm(pb, pg, ppt))
                if i >= 2:
                    qb, qg, qstates = hist[i - 2]
                    emit_norm(qb, qg, qstates)
                hist.append((b, g, pt))
            n = len(pairs)
            pb, pg, ppt = hist[n - 1]
            emit_norm(*hist[n - 2])
            emit_norm(pb, pg, emit_csot_mm(pb, pg, ppt))

        # ---- phase D: output projection ----
        with ExitStack() as phase3:
            opool = phase3.enter_context(tc.tile_pool(name="opool", bufs=3))
            psD = phase3.enter_context(
                tc.tile_pool(name="psD", bufs=3, space=bass.MemorySpace.PSUM))
            NCH = HID // T  # 2 chunks of 512
            for tt in range(NTOK_T):
                o_tile = opool.tile([P, HID], BF16, tag="o")
                # interleave both output chunks k-major: consecutive matmul
                # pairs share the stationary operand OT[:,k,tt-block]
                ps0 = psD.tile([P, T], FP32, tag="dps0")
                ps1 = psD.tile([P, T], FP32, tag="dps1")
                pss = [ps0, ps1]
                for k in range(KT_HID):
                    for cchunk in range(NCH):
                        nc.tensor.matmul(
                            pss[cchunk],
                            ot_sb[:, k, tt * P:(tt + 1) * P],
                            wo_sb[:, k, cchunk * T:(cchunk + 1) * T],
                            start=(k == 0), stop=(k == KT_HID - 1))
                # alternate engines so the copies run in parallel
                nc.vector.tensor_copy(o_tile[:, 0:T], pss[0])
                nc.scalar.copy(o_tile[:, T:HID], pss[1])
                eng = nc.sync if tt % 2 == 0 else nc.scalar
                eng.dma_start(out=out[tt * P:(tt + 1) * P, :], in_=o_tile)


_COMPILED = None


def _get_compiled():
    global _COMPILED
    if _COMPILED is None:
        nc = bacc.Bacc("TRN2", target_bir_lowering=False, debug=False)
        _build(nc)
        nc.compile()
        _COMPILED = nc
    return _COMPILED


def kernel(hidden_states, Wq, Wk, Wv, Wo, _trace=False, _trace_kwargs=None):
    hs = np.asarray(hidden_states, dtype=np.float32).astype(BF)
    wq = np.ascontiguousarray(np.asarray(Wq, dtype=np.float32).astype(BF))
    wk = np.ascontiguousarray(np.asarray(Wk, dtype=np.float32).astype(BF))
    wv = np.ascontiguousarray(np.asarray(Wv, dtype=np.float32).astype(BF))
    wo = np.ascontiguousarray(np.asarray(Wo, dtype=np.float32).astype(BF))
    consts = _host_consts()
    nc = _get_compiled()
    in_maps = []
    for c in range(NCORES):
        # ship X pre-transposed ([HID, TOK]) so the kernel's lhs/rhs layouts
        # need no on-chip transpose of X at all
        shard_t = np.ascontiguousarray(
            hs[BL * c: BL * (c + 1)].reshape(TOK, HID).T)
        in_maps.append({"hidden_t": shard_t, "Wq": wq, "Wk": wk, "Wv": wv,
                        "Wo": wo, **consts})
    res = run_bass_kernel_spmd(
        nc, in_maps, list(range(NCORES)), trace=_trace,
        **(_trace_kwargs or {}))
    outs = [r["out"].astype(np.float32).reshape(BL, T, HID)
            for r in res.results]
    full = np.concatenate(outs, axis=0)
    if _trace:
        return full, res
    return full


# revision 17
# speedup vs baseline: 1.0781x; 1.0781x over previous
"""GQA attention kernel for Trainium2, data-parallel over batch on 8 NeuronCores.

Per-core problem (2 of 16 batches): X [1024tok, 1024] -> QKV proj -> RoPE ->
causal GQA attention (8 q heads, 4 kv heads, D=128) -> out proj [1024, 1024].

Layout strategy: everything stays in "feature-on-partition" transposed form,
and attention scores are computed TRANSPOSED (ST[tk,tq]) so that exp(ST) is
already the P.T the PV matmul needs -- no transposes of P at all. Matmul
operands are bf16 (fp32 PSUM accumulate).

Schedule (all per core):
  XT[hid,tok]   = host-pretransposed X                     (DRAM -> SBUF)
  QT[dq,tok]    = Wq.T @ XT   k-outer waves of 8 chains  + RoPE
  KT[dkv,tok]   = Wk.T @ XT   + RoPE
  V [tok,dkv]   = X @ Wv      (lhsT = XT, rhs = Wv)
  attention per (batch, kv-group) head PAIR, software-pipelined one pair deep:
    ST[tk, 2, tq] = KT_j.T @ QT_h  per 128-row tk block, causal col range,
                    both heads of the group into one 2-bank psum tile
    PT            = exp(ST)  one ACT op per (pair, j) via 3D AP
    mask          = one DVE mul per (pair, j) over both heads' diag blocks
                    (stride-0 broadcast mask operand)
    colsum[1,tq] += ones[128,1].T @ PT_j    (PE)
    OT[d,tq]     += V_j.T @ PT_j            (PE accumulate over j)
    norm per head, pipelined entirely off the PE:
      cs row -> scratch-DRAM roundtrip transpose -> WIDE reciprocal [128,4]
      -> roundtrip back -> DMA partition-broadcast [128,T] -> in-place
      SBUF multiply of the already-copied-out OT
  Out[tok,hid]  = OT.T @ Wo   -> bf16 store (host casts back to fp32)
RoPE scale 1/sqrt(D) is folded into the Q cos/sin host constants.
"""

import numpy as np
import ml_dtypes
from contextlib import ExitStack

import concourse.bass as bass
import concourse.tile as tile
from concourse import bacc, mybir
from concourse.bass_utils import run_bass_kernel_spmd

B, T, HID = 16, 512, 1024
NH, NKV, D = 8, 4, 128
THETA = 10000.0
NCORES = 8
BL = B // NCORES          # local batches per core
TOK = BL * T              # local tokens
P = 128
KT_HID = HID // P         # 8 contraction tiles over hidden
NTQ = T // P              # 4 tk/tq tiles per sequence
GROUPS = NH // NKV        # 2 q heads per kv head
NTOK_T = TOK // P         # 8 token tiles per core
FP32 = mybir.dt.float32
BF16 = mybir.dt.bfloat16
BF = ml_dtypes.bfloat16


def _host_consts():
    inv_freq = 1.0 / (THETA ** (np.arange(0, D, 2, dtype=np.float64) / D))
    freqs = np.outer(np.arange(T, dtype=np.float64), inv_freq)    # [T, 64]
    emb = np.concatenate([freqs, freqs], axis=-1)                 # [T, 128]
    cos = np.cos(emb).T                                           # [128, T]
    sin = np.sin(emb).T
    scale = 1.0 / np.sqrt(D)
    # rotate_half sign folded into sin: out = q*cos + qswap*sin_signed where
    # qswap is q with its partition halves swapped
    sin_signed = np.concatenate([-sin[:D // 2], sin[D // 2:]], axis=0)
    # transposed-S diagonal-block multiplicative mask: rows tk, cols tq;
    # valid iff tq >= tk
    mask_t = np.triu(np.ones((P, P), np.float32)).astype(BF)
    return {
        "cos_q": (cos * scale).astype(BF),
        "sin_q": (sin_signed * scale).astype(BF),
        "cos_k": cos.astype(BF),
        "sin_k": sin_signed.astype(BF),
        "mask_t": mask_t,
    }


def _rope(nc, out_sl, psum, cos_sb, sin_sb, tmp_pool):
    """out = q * cos + rotate_half(q) * sin for q = psum, all [128, T] APs.

    One ACT copy moves psum -> bf16 SBUF (single slow PSUM read), then the
    arithmetic runs in the DVE's fast bf16-SBUF mode.
    """
    H = D // 2
    qraw = tmp_pool.tile([P, T], BF16, tag="rope_raw")
    nc.scalar.copy(qraw, psum)
    # partition-half swap of rotate_half runs on the DMA engine (compute
    # engines cannot shift partitions between SBUF operands); the sign of
    # rotate_half is folded into the host sin constant
    qswap = tmp_pool.tile([P, T], BF16, tag="rope_swap")
    nc.sync.dma_start(out=qswap[0:H], in_=qraw[H:P])
    nc.sync.dma_start(out=qswap[H:P], in_=qraw[0:H])
    tmp = tmp_pool.tile([P, T], BF16, tag="rope_tmp")
    nc.gpsimd.tensor_mul(tmp, qswap, sin_sb)
    nc.vector.tensor_mul(out_sl, qraw, cos_sb)
    nc.vector.tensor_add(out_sl, out_sl, tmp)


def _build(nc):
    # hidden arrives pre-transposed from the host: [HID, TOK]
    hid_t = nc.dram_tensor("hidden_t", [HID, TOK], BF16,
                           kind="ExternalInput").ap()
    wq = nc.dram_tensor("Wq", [HID, NH * D], BF16, kind="ExternalInput").ap()
    wk = nc.dram_tensor("Wk", [HID, NKV * D], BF16, kind="ExternalInput").ap()
    wv = nc.dram_tensor("Wv", [HID, NKV * D], BF16, kind="ExternalInput").ap()
    wo = nc.dram_tensor("Wo", [NH * D, HID], BF16, kind="ExternalInput").ap()
    cos_q = nc.dram_tensor("cos_q", [P, T], BF16, kind="ExternalInput").ap()
    sin_q = nc.dram_tensor("sin_q", [P, T], BF16, kind="ExternalInput").ap()
    cos_k = nc.dram_tensor("cos_k", [P, T], BF16, kind="ExternalInput").ap()
    sin_k = nc.dram_tensor("sin_k", [P, T], BF16, kind="ExternalInput").ap()
    mask_t = nc.dram_tensor("mask_t", [P, P], BF16, kind="ExternalInput").ap()
    out = nc.dram_tensor("out", [TOK, HID], BF16, kind="ExternalOutput").ap()
    # scratch rows for the tiny row<->column transposes of the softmax
    # denominators (1KB per head; distinct rows avoid WAR between pipelined
    # head chains). All hops ride the sync DMA queue, which serializes them
    # without Tile needing to track the DRAM aliasing.
    cs_dram = nc.dram_tensor("cs_scratch", [BL * NH, T], BF16,
                             kind="Internal").ap()
    ri_dram = nc.dram_tensor("ri_scratch", [BL * NH, T], BF16,
                             kind="Internal").ap()

    with tile.TileContext(nc) as tc, ExitStack() as ctx:
        # ---- pools with cross-phase lifetimes ----
        consts = ctx.enter_context(tc.tile_pool(name="consts", bufs=1))

        cosq_sb = consts.tile([P, T], BF16, tag="cq")
        sinq_sb = consts.tile([P, T], BF16, tag="sq")
        cosk_sb = consts.tile([P, T], BF16, tag="ck")
        sink_sb = consts.tile([P, T], BF16, tag="sk")
        maskt_sb = consts.tile([P, P], BF16, tag="maskt")
        ones_bf = consts.tile([P, P], BF16, tag="ones")
        warm_rhs = consts.tile([P, 256], BF16, tag="warm")
        # memsets on the (otherwise idle at startup) gpsimd engine so the
        # PE warmup can begin as early as possible
        nc.gpsimd.memset(ones_bf, 1.0)
        nc.gpsimd.memset(warm_rhs, 0.0)

        qkvpool = ctx.enter_context(tc.tile_pool(name="qkv", bufs=1))
        qt_sb = qkvpool.tile([P, NH, TOK], BF16, tag="qt")     # [d, h, tok]
        kt_sb = qkvpool.tile([P, NKV, TOK], BF16, tag="kt")    # [d, g, tok]
        v_sb = qkvpool.tile([P, NTOK_T, NKV * D], BF16, tag="v")  # [tok,tt,dkv]

        # ---- phase A+B: loads + QKV projections (k-outer waves) ----
        with ExitStack() as phase1:
            wpool = phase1.enter_context(tc.tile_pool(name="wpool", bufs=1))
            xtp = phase1.enter_context(tc.tile_pool(name="xtp", bufs=1))
            ropet = phase1.enter_context(tc.tile_pool(name="ropet", bufs=4))
            psB = phase1.enter_context(
                tc.tile_pool(name="psB", bufs=8, space=bass.MemorySpace.PSUM))

            wq_sb = wpool.tile([P, KT_HID, NH * D], BF16, tag="wq")
            wk_sb = wpool.tile([P, KT_HID, NKV * D], BF16, tag="wk")
            wv_sb = wpool.tile([P, KT_HID, NKV * D], BF16, tag="wv")
            xt_sb = xtp.tile([P, KT_HID, TOK], BF16, tag="xt")  # [hid, k, tok]
            wq_r = wq.rearrange("(k p) n -> p k n", p=P)
            wk_r = wk.rearrange("(k p) n -> p k n", p=P)
            wv_r = wv.rearrange("(k p) n -> p k n", p=P)
            hid_r = hid_t.rearrange("(k p) t -> p k t", p=P)
            # load order follows consumption order: the Q projection streams
            # k-chunk by k-chunk, so (xt[k], wq[k]) pairs go first, split
            # across the two HARDWARE DGE rings (sync + scalar; the gpsimd
            # ring is software DGE executed on the Q7 cores -- never use it
            # for bulk). RoPE consts next; then wk/wv/wo.
            for k in range(KT_HID):
                nc.sync.dma_start(out=xt_sb[:, k, :], in_=hid_r[:, k, :])
                nc.scalar.dma_start(out=wq_sb[:, k, :], in_=wq_r[:, k, :])
            nc.sync.dma_start(out=cosq_sb, in_=cos_q)
            nc.sync.dma_start(out=sinq_sb, in_=sin_q)
            nc.scalar.dma_start(out=cosk_sb, in_=cos_k)
            nc.scalar.dma_start(out=sink_sb, in_=sin_k)
            nc.scalar.dma_start(out=maskt_sb, in_=mask_t)
            for k in range(KT_HID):
                nc.sync.dma_start(out=wk_sb[:, k, :], in_=wk_r[:, k, :])
                nc.scalar.dma_start(out=wv_sb[:, k, :], in_=wv_r[:, k, :])

            # PE warmup: ~3.5us of dependency-light matmuls ahead of the
            # first projection so the HAM clock-gate releases (1.2 ->
            # 2.4 GHz) while the input DMAs are still in flight
            wps = psB.tile([P, T], FP32, tag="projps")
            for _ in range(16):
                nc.tensor.matmul(wps[:, 0:256], ones_bf, warm_rhs,
                                 start=True, stop=True, skip_group_check=True)

            def q_wave(c):
                # k-outer: the first matmuls need only (xt[0], wq[0]), so
                # the PE starts when the first 512KB lands, not after the
                # whole load; DMA delivery and PE consumption are balanced.
                pss = [psB.tile([P, T], FP32, tag="projps",
                                name=f"psq{c}_{i}") for i in range(NH)]
                for k in range(KT_HID):
                    for h in range(NH):
                        nc.tensor.matmul(
                            pss[h],
                            wq_sb[:, k, h * P:(h + 1) * P],
                            xt_sb[:, k, c * T:(c + 1) * T],
                            start=(k == 0), stop=(k == KT_HID - 1))
                for h in range(NH):
                    _rope(nc, qt_sb[:, h, c * T:(c + 1) * T], pss[h],
                          cosq_sb, sinq_sb, ropet)

            def k_wave():
                chains = [(g, cc) for g in range(NKV) for cc in range(BL)]
                pss = [psB.tile([P, T], FP32, tag="projps", name=f"psk{i}")
                       for i in range(len(chains))]
                for k in range(KT_HID):
                    for i, (g, cc) in enumerate(chains):
                        nc.tensor.matmul(
                            pss[i],
                            wk_sb[:, k, g * P:(g + 1) * P],
                            xt_sb[:, k, cc * T:(cc + 1) * T],
                            start=(k == 0), stop=(k == KT_HID - 1))
                for i, (g, cc) in enumerate(chains):
                    _rope(nc, kt_sb[:, g, cc * T:(cc + 1) * T], pss[i],
                          cosk_sb, sink_sb, ropet)

            def v_wave():
                pss = [psB.tile([P, T], FP32, tag="projps", name=f"psv{i}")
                       for i in range(NTOK_T)]
                for k in range(KT_HID):
                    for tt in range(NTOK_T):
                        nc.tensor.matmul(
                            pss[tt][:, :NKV * D],
                            xt_sb[:, k, tt * P:(tt + 1) * P],
                            wv_sb[:, k, :],
                            start=(k == 0), stop=(k == KT_HID - 1))
                for tt in range(NTOK_T):
                    # alternate copy engines so the drain is not ACT-serial
                    if tt % 2 == 0:
                        nc.scalar.copy(v_sb[:, tt, :], pss[tt][:, :NKV * D])
                    else:
                        nc.vector.tensor_copy(v_sb[:, tt, :],
                                              pss[tt][:, :NKV * D])

            q_wave(0)
            k_wave()
            v_wave()
            q_wave(1)

        # ---- phase C: attention, head pairs, one-pair software pipeline ----
        otpool = ctx.enter_context(tc.tile_pool(name="otpool", bufs=1))
        ot_sb = otpool.tile([P, NH, TOK], BF16, tag="ot")      # [d, h, tok]
        wopool = ctx.enter_context(tc.tile_pool(name="wopool", bufs=1))
        wo_sb = wopool.tile([P, KT_HID, HID], BF16, tag="wo")
        nc.sync.dma_start(out=wo_sb, in_=wo.rearrange("(k p) n -> p k n", p=P))

        with ExitStack() as phase2:
            ptpool = phase2.enter_context(tc.tile_pool(name="ptpool", bufs=2))
            rowp = phase2.enter_context(tc.tile_pool(name="rowp", bufs=6))
            # PSUM budget is exactly 8 banks: ST pair tile 2 (bufs=1 is
            # enough -- by emission order the PE runs CSOT(p-1) between
            # ST(p) and ST(p+1), so exp(p) has long drained the tile) +
            # o_ps 2 + cs 2 + rank-1 broadcast 2.
            psS = phase2.enter_context(
                tc.tile_pool(name="psS", bufs=1, space=bass.MemorySpace.PSUM))
            psO = phase2.enter_context(
                tc.tile_pool(name="psO", bufs=2, space=bass.MemorySpace.PSUM))
            psC = phase2.enter_context(
                tc.tile_pool(name="psC", bufs=2, space=bass.MemorySpace.PSUM))
            psR = phase2.enter_context(
                tc.tile_pool(name="psR", bufs=2, space=bass.MemorySpace.PSUM))

            mask_b = maskt_sb[:, None, :].to_broadcast((P, GROUPS, P))

            def emit_st(b, g):
                """ST matmuls + exp + mask for one head pair; returns pt."""
                st = psS.tile([P, GROUPS, T], FP32, tag="sps")
                pt = ptpool.tile([P, GROUPS, NTQ, T], BF16, tag="pt")
                for j in range(NTQ):
                    lo = j * P
                    for hh in range(GROUPS):
                        h = GROUPS * g + hh
                        nc.tensor.matmul(
                            st[:, hh, lo:T],
                            kt_sb[:, g, b * T + lo: b * T + lo + P],
                            qt_sb[:, h, b * T + lo: (b + 1) * T],
                            start=True, stop=True)
                    # one exp per (pair, j): 3D AP spanning both psum banks
                    # (no row-max: logits are O(1) by construction)
                    nc.scalar.activation(
                        out=pt[:, :, j, lo:T], in_=st[:, :, lo:T],
                        func=mybir.ActivationFunctionType.Exp,
                        bias=0.0, scale=1.0)
                    # causal mask on the diagonal block, both heads in one
                    # op (mask operand broadcast along the head axis), on
                    # the otherwise-idle gpsimd
                    nc.gpsimd.tensor_mul(
                        pt[:, :, j, lo:lo + P], pt[:, :, j, lo:lo + P],
                        mask_b)
                return pt

            def emit_csot_mm(b, g, pt):
                """colsum + OT accumulation; denominator row shipped out.

                Returns per-head state for the deferred normalization."""
                states = []
                for hh in range(GROUPS):
                    h = GROUPS * g + hh
                    row = b * NH + h
                    o_ps = psO.tile([P, T], FP32, tag="ops")
                    cs_ps = psC.tile([1, T], FP32, tag="cps")
                    # colsum matmuls first so the denominator DMA chain
                    # launches before the OT matmuls run
                    for j in range(NTQ):
                        lo = j * P
                        nc.tensor.matmul(
                            cs_ps[:, lo:T] if j else cs_ps[:, :],
                            ones_bf[:, 0:1],
                            pt[:, hh, j, lo:T],
                            start=(j == 0), stop=(j == NTQ - 1),
                            skip_group_check=True)
                    # denominator off to scratch DRAM: the [1,T] row is
                    # lane-starved on every compute engine (a [1,512]
                    # reciprocal costs ~3.4us!), so roundtrip through DRAM
                    # to transpose it into [128, 4] columns. Hops ride the
                    # sync queue, which orders the DRAM write->read.
                    csrow = rowp.tile([1, T], BF16, tag="csrow")
                    if hh == 0:
                        nc.scalar.copy(csrow, cs_ps)
                    else:
                        nc.vector.tensor_copy(csrow, cs_ps)
                    nc.sync.dma_start(out=cs_dram[row:row + 1, :], in_=csrow)
                    ccol = rowp.tile([P, NTQ], BF16, tag="ccol")
                    nc.sync.dma_start(
                        out=ccol,
                        in_=cs_dram[row].rearrange("(q p) -> p q", p=P))
                    for j in range(NTQ):
                        lo = j * P
                        nc.tensor.matmul(
                            o_ps[:, lo:T] if j else o_ps[:, :],
                            v_sb[:, b * NTQ + j, g * D:(g + 1) * D],
                            pt[:, hh, j, lo:T],
                            start=(j == 0), stop=(j == NTQ - 1),
                            skip_group_check=True)
                    # unnormalized OT out of PSUM immediately (frees banks;
                    # the normalization multiply lands later, in-place)
                    nc.vector.tensor_copy(
                        ot_sb[:, h, b * T:(b + 1) * T], o_ps)
                    states.append((h, row, ccol))
                return states

            def emit_recip(states):
                """wide reciprocal + roundtrip back to row form.

                One pipeline stage after emit_csot_mm: by now the two DMA
                hops that transposed the colsum row into [128, NTQ] columns
                have landed, so the in-order DVE queue doesn't stall."""
                out_states = []
                for h, row, ccol in states:
                    rcol = rowp.tile([P, NTQ], BF16, tag="rcol")
                    with nc.allow_low_precision("bf16 softmax denominator"):
                        nc.vector.reciprocal(rcol, ccol)
                    nc.sync.dma_start(
                        out=ri_dram[row].rearrange("(q p) -> p q", p=P),
                        in_=rcol)
                    rrow = rowp.tile([1, T], BF16, tag="rrow")
                    nc.sync.dma_start(out=rrow,
                                      in_=ri_dram[row:row + 1, :])
                    out_states.append((h, rrow))
                return out_states

            def emit_apply(b, g, states):
                """rank-1 broadcast + in-place multiply, one stage later.

                The full chain (4 DMA hops + reciprocal) thus gets ~2 pair
                periods to complete before the PE reaches the rank-1 -- the
                PE must never wait, else HAM re-throttles it to 1.2 GHz."""
                for h, rrow in states:
                    # broadcast the reciprocal row to all partitions with a
                    # warm rank-1 matmul (~250ns; a DMA partition-broadcast
                    # of [128,T] costs ~8us in replicated descriptors!)
                    rb_ps = psR.tile([P, T], FP32, tag="rbps")
                    nc.tensor.matmul(rb_ps, ones_bf[0:1, :], rrow,
                                     start=True, stop=True,
                                     skip_group_check=True)
                    nc.vector.tensor_mul(
                        ot_sb[:, h, b * T:(b + 1) * T],
                        ot_sb[:, h, b * T:(b + 1) * T], rb_ps)

            pairs = [(b, g) for b in range(BL) for g in range(NKV)]
            hist = []  # per pair: (b, g, payload) pipeline registers
            for i, (b, g) in enumerate(pairs):
                pt = emit_st(b, g)
                if i >= 1:
                    pb, pg, ppt = hist[i - 1]
                    hist[i - 1] = (pb, pg, emit_csot_mm(pb, pg, ppt))
                if i >= 2:
                    qb, qg, qs = hist[i - 2]
                    hist[i - 2] = (qb, qg, emit_recip(qs))
                if i >= 3:
                    rb_, rg_, rs = hist[i - 3]
                    emit_apply(rb_, rg_, rs)
                hist.append((b, g, pt))
            n = len(pairs)
            pb, pg, ppt = hist[n - 1]
            hist[n - 1] = (pb, pg, emit_csot_mm(pb, pg, ppt))
            hist[n - 2] = (hist[n - 2][0], hist[n - 2][1],
                           emit_recip(hist[n - 2][2]))
            emit_apply(*hist[n - 3])
            hist[n - 1] = (pb, pg, emit_recip(hist[n - 1][2]))
            emit_apply(*hist[n - 2])
            emit_apply(*hist[n - 1])

        # ---- phase D: output projection ----
        with ExitStack() as phase3:
            opool = phase3.enter_context(tc.tile_pool(name="opool", bufs=3))
            psD = phase3.enter_context(
                tc.tile_pool(name="psD", bufs=3, space=bass.MemorySpace.PSUM))
            NCH = HID // T  # 2 chunks of 512
            for tt in range(NTOK_T):
                o_tile = opool.tile([P, HID], BF16, tag="o")
                # interleave both output chunks k-major: consecutive matmul
                # pairs share the stationary operand OT[:,k,tt-block]
                ps0 = psD.tile([P, T], FP32, tag="dps0")
                ps1 = psD.tile([P, T], FP32, tag="dps1")
                pss = [ps0, ps1]
                for k in range(KT_HID):
                    for cchunk in range(NCH):
                        nc.tensor.matmul(
                            pss[cchunk],
                            ot_sb[:, k, tt * P:(tt + 1) * P],
                            wo_sb[:, k, cchunk * T:(cchunk + 1) * T],
                            start=(k == 0), stop=(k == KT_HID - 1))
                # alternate engines so the copies run in parallel
                nc.vector.tensor_copy(o_tile[:, 0:T], pss[0])
                nc.scalar.copy(o_tile[:, T:HID], pss[1])
                eng = nc.sync if tt % 2 == 0 else nc.scalar
                eng.dma_start(out=out[tt * P:(tt + 1) * P, :], in_=o_tile)


_COMPILED = None


def _get_compiled():
    global _COMPILED
    if _COMPILED is None:
        nc = bacc.Bacc("TRN2", target_bir_lowering=False, debug=False)
        _build(nc)
        nc.compile()
        _COMPILED = nc
    return _COMPILED


def kernel(hidden_states, Wq, Wk, Wv, Wo, _trace=False, _trace_kwargs=None):
    hs = np.asarray(hidden_states, dtype=np.float32).astype(BF)
    wq = np.ascontiguousarray(np.asarray(Wq, dtype=np.float32).astype(BF))
    wk = np.ascontiguousarray(np.asarray(Wk, dtype=np.float32).astype(BF))
    wv = np.ascontiguousarray(np.asarray(Wv, dtype=np.float32).astype(BF))
    wo = np.ascontiguousarray(np.asarray(Wo, dtype=np.float32).astype(BF))
    consts = _host_consts()
    nc = _get_compiled()
    in_maps = []
    for c in range(NCORES):
        # ship X pre-transposed ([HID, TOK]) so the kernel's lhs/rhs layouts
        # need no on-chip transpose of X at all
        shard_t = np.ascontiguousarray(
            hs[BL * c: BL * (c + 1)].reshape(TOK, HID).T)
        in_maps.append({"hidden_t": shard_t, "Wq": wq, "Wk": wk, "Wv": wv,
                        "Wo": wo, **consts})
    res = run_bass_kernel_spmd(
        nc, in_maps, list(range(NCORES)), trace=_trace,
        **(_trace_kwargs or {}))
    outs = [r["out"].astype(np.float32).reshape(BL, T, HID)
            for r in res.results]
    full = np.concatenate(outs, axis=0)
    if _trace:
        return full, res
    return full


# revision 20
# speedup vs baseline: 1.7552x; 1.6280x over previous
"""GQA attention kernel for Trainium2, data-parallel over batch on 8 NeuronCores.

Per-core problem (2 of 16 batches): X [1024tok, 1024] -> QKV proj -> RoPE ->
causal GQA attention (8 q heads, 4 kv heads, D=128) -> out proj [1024, 1024].

Layout strategy: everything stays in "feature-on-partition" transposed form,
and attention scores are computed TRANSPOSED (ST[tk,tq]) so that exp(ST) is
already the P.T the PV matmul needs -- no transposes of P at all. Matmul
operands are bf16 (fp32 PSUM accumulate).

Schedule (all per core):
  XT[hid,tok]   = host-pretransposed X                     (DRAM -> SBUF)
  QT[dq,tok]    = Wq.T @ XT   k-outer waves of 8 chains  + RoPE
  KT[dkv,tok]   = Wk.T @ XT   + RoPE
  V [tok,dkv]   = X @ Wv      (lhsT = XT, rhs = Wv)
  attention per (batch, kv-group) head PAIR, software-pipelined one pair deep:
    ST[tk, 2, tq] = KT_j.T @ QT_h  per 128-row tk block, causal col range,
                    both heads of the group into one 2-bank psum tile
    PT            = exp(ST)  one ACT op per (pair, j) via 3D AP
    mask          = one DVE mul per (pair, j) over both heads' diag blocks
                    (stride-0 broadcast mask operand)
    colsum[1,tq] += ones[128,1].T @ PT_j    (PE)
    OT[d,tq]     += V_j.T @ PT_j            (PE accumulate over j)
    norm per head, pipelined entirely off the PE:
      cs row -> scratch-DRAM roundtrip transpose -> WIDE reciprocal [128,4]
      -> roundtrip back -> DMA partition-broadcast [128,T] -> in-place
      SBUF multiply of the already-copied-out OT
  Out[tok,hid]  = OT.T @ Wo   -> bf16 store (host casts back to fp32)
RoPE scale 1/sqrt(D) is folded into the Q cos/sin host constants.
"""

import numpy as np
import ml_dtypes
from contextlib import ExitStack

import concourse.bass as bass
import concourse.tile as tile
from concourse import bacc, mybir
from concourse.bass_utils import run_bass_kernel_spmd

B, T, HID = 16, 512, 1024
NH, NKV, D = 8, 4, 128
THETA = 10000.0
NCORES = 8
BL = B // NCORES          # local batches per core
TOK = BL * T              # local tokens
P = 128
KT_HID = HID // P         # 8 contraction tiles over hidden
NTQ = T // P              # 4 tk/tq tiles per sequence
GROUPS = NH // NKV        # 2 q heads per kv head
NTOK_T = TOK // P         # 8 token tiles per core
FP32 = mybir.dt.float32
BF16 = mybir.dt.bfloat16
BF = ml_dtypes.bfloat16


def _host_consts():
    inv_freq = 1.0 / (THETA ** (np.arange(0, D, 2, dtype=np.float64) / D))
    freqs = np.outer(np.arange(T, dtype=np.float64), inv_freq)    # [T, 64]
    emb = np.concatenate([freqs, freqs], axis=-1)                 # [T, 128]
    cos = np.cos(emb).T                                           # [128, T]
    sin = np.sin(emb).T
    scale = 1.0 / np.sqrt(D)
    # rotate_half sign folded into sin: out = q*cos + qswap*sin_signed where
    # qswap is q with its partition halves swapped
    sin_signed = np.concatenate([-sin[:D // 2], sin[D // 2:]], axis=0)
    # transposed-S diagonal-block multiplicative mask: rows tk, cols tq;
    # valid iff tq >= tk
    mask_t = np.triu(np.ones((P, P), np.float32)).astype(BF)
    return {
        "cos_q": (cos * scale).astype(BF),
        "sin_q": (sin_signed * scale).astype(BF),
        "cos_k": cos.astype(BF),
        "sin_k": sin_signed.astype(BF),
        "mask_t": mask_t,
    }


def _rope(nc, out_sl, psum, cos_sb, sin_sb, tmp_pool):
    """out = q * cos + rotate_half(q) * sin for q = psum, all [128, T] APs.

    One ACT copy moves psum -> bf16 SBUF (single slow PSUM read), then the
    arithmetic runs in the DVE's fast bf16-SBUF mode.
    """
    H = D // 2
    qraw = tmp_pool.tile([P, T], BF16, tag="rope_raw")
    nc.scalar.copy(qraw, psum)
    # partition-half swap of rotate_half runs on the DMA engine (compute
    # engines cannot shift partitions between SBUF operands); the sign of
    # rotate_half is folded into the host sin constant
    qswap = tmp_pool.tile([P, T], BF16, tag="rope_swap")
    nc.sync.dma_start(out=qswap[0:H], in_=qraw[H:P])
    nc.sync.dma_start(out=qswap[H:P], in_=qraw[0:H])
    tmp = tmp_pool.tile([P, T], BF16, tag="rope_tmp")
    nc.gpsimd.tensor_mul(tmp, qswap, sin_sb)
    nc.vector.tensor_mul(out_sl, qraw, cos_sb)
    nc.vector.tensor_add(out_sl, out_sl, tmp)


def _build(nc):
    # hidden arrives pre-transposed from the host: [HID, TOK]
    hid_t = nc.dram_tensor("hidden_t", [HID, TOK], BF16,
                           kind="ExternalInput").ap()
    wq = nc.dram_tensor("Wq", [HID, NH * D], BF16, kind="ExternalInput").ap()
    wk = nc.dram_tensor("Wk", [HID, NKV * D], BF16, kind="ExternalInput").ap()
    wv = nc.dram_tensor("Wv", [HID, NKV * D], BF16, kind="ExternalInput").ap()
    wo = nc.dram_tensor("Wo", [NH * D, HID], BF16, kind="ExternalInput").ap()
    cos_q = nc.dram_tensor("cos_q", [P, T], BF16, kind="ExternalInput").ap()
    sin_q = nc.dram_tensor("sin_q", [P, T], BF16, kind="ExternalInput").ap()
    cos_k = nc.dram_tensor("cos_k", [P, T], BF16, kind="ExternalInput").ap()
    sin_k = nc.dram_tensor("sin_k", [P, T], BF16, kind="ExternalInput").ap()
    mask_t = nc.dram_tensor("mask_t", [P, P], BF16, kind="ExternalInput").ap()
    out = nc.dram_tensor("out", [TOK, HID], BF16, kind="ExternalOutput").ap()

    with tile.TileContext(nc) as tc, ExitStack() as ctx:
        # ---- pools with cross-phase lifetimes ----
        consts = ctx.enter_context(tc.tile_pool(name="consts", bufs=1))

        cosq_sb = consts.tile([P, T], BF16, tag="cq")
        sinq_sb = consts.tile([P, T], BF16, tag="sq")
        cosk_sb = consts.tile([P, T], BF16, tag="ck")
        sink_sb = consts.tile([P, T], BF16, tag="sk")
        maskt_sb = consts.tile([P, P], BF16, tag="maskt")
        ones_bf = consts.tile([P, P], BF16, tag="ones")
        warm_rhs = consts.tile([P, 256], BF16, tag="warm")
        # memsets on the (otherwise idle at startup) gpsimd engine so the
        # PE warmup can begin as early as possible
        nc.gpsimd.memset(ones_bf, 1.0)
        nc.gpsimd.memset(warm_rhs, 0.0)

        qkvpool = ctx.enter_context(tc.tile_pool(name="qkv", bufs=1))
        qt_sb = qkvpool.tile([P, NH, TOK], BF16, tag="qt")     # [d, h, tok]
        kt_sb = qkvpool.tile([P, NKV, TOK], BF16, tag="kt")    # [d, g, tok]
        v_sb = qkvpool.tile([P, NTOK_T, NKV * D], BF16, tag="v")  # [tok,tt,dkv]

        # ---- phase A+B: loads + QKV projections (k-outer waves) ----
        with ExitStack() as phase1:
            wpool = phase1.enter_context(tc.tile_pool(name="wpool", bufs=1))
            xtp = phase1.enter_context(tc.tile_pool(name="xtp", bufs=1))
            ropet = phase1.enter_context(tc.tile_pool(name="ropet", bufs=4))
            psB = phase1.enter_context(
                tc.tile_pool(name="psB", bufs=8, space=bass.MemorySpace.PSUM))

            wq_sb = wpool.tile([P, KT_HID, NH * D], BF16, tag="wq")
            wk_sb = wpool.tile([P, KT_HID, NKV * D], BF16, tag="wk")
            wv_sb = wpool.tile([P, KT_HID, NKV * D], BF16, tag="wv")
            xt_sb = xtp.tile([P, KT_HID, TOK], BF16, tag="xt")  # [hid, k, tok]
            wq_r = wq.rearrange("(k p) n -> p k n", p=P)
            wk_r = wk.rearrange("(k p) n -> p k n", p=P)
            wv_r = wv.rearrange("(k p) n -> p k n", p=P)
            hid_r = hid_t.rearrange("(k p) t -> p k t", p=P)
            # load order follows consumption order: the Q projection streams
            # k-chunk by k-chunk, so (xt[k], wq[k]) pairs go first, split
            # across the two HARDWARE DGE rings (sync + scalar; the gpsimd
            # ring is software DGE executed on the Q7 cores -- never use it
            # for bulk). RoPE consts next; then wk/wv/wo.
            for k in range(KT_HID):
                nc.sync.dma_start(out=xt_sb[:, k, :], in_=hid_r[:, k, :])
                nc.scalar.dma_start(out=wq_sb[:, k, :], in_=wq_r[:, k, :])
            nc.sync.dma_start(out=cosq_sb, in_=cos_q)
            nc.sync.dma_start(out=sinq_sb, in_=sin_q)
            nc.scalar.dma_start(out=cosk_sb, in_=cos_k)
            nc.scalar.dma_start(out=sink_sb, in_=sin_k)
            nc.scalar.dma_start(out=maskt_sb, in_=mask_t)
            for k in range(KT_HID):
                nc.sync.dma_start(out=wk_sb[:, k, :], in_=wk_r[:, k, :])
                nc.scalar.dma_start(out=wv_sb[:, k, :], in_=wv_r[:, k, :])

            # PE warmup: ~3.5us of dependency-light matmuls ahead of the
            # first projection so the HAM clock-gate releases (1.2 ->
            # 2.4 GHz) while the input DMAs are still in flight
            wps = psB.tile([P, T], FP32, tag="projps")
            for _ in range(16):
                nc.tensor.matmul(wps[:, 0:256], ones_bf, warm_rhs,
                                 start=True, stop=True, skip_group_check=True)

            def q_wave(c):
                # k-outer: the first matmuls need only (xt[0], wq[0]), so
                # the PE starts when the first 512KB lands, not after the
                # whole load; DMA delivery and PE consumption are balanced.
                pss = [psB.tile([P, T], FP32, tag="projps",
                                name=f"psq{c}_{i}") for i in range(NH)]
                for k in range(KT_HID):
                    for h in range(NH):
                        nc.tensor.matmul(
                            pss[h],
                            wq_sb[:, k, h * P:(h + 1) * P],
                            xt_sb[:, k, c * T:(c + 1) * T],
                            start=(k == 0), stop=(k == KT_HID - 1))
                for h in range(NH):
                    _rope(nc, qt_sb[:, h, c * T:(c + 1) * T], pss[h],
                          cosq_sb, sinq_sb, ropet)

            def k_wave():
                chains = [(g, cc) for g in range(NKV) for cc in range(BL)]
                pss = [psB.tile([P, T], FP32, tag="projps", name=f"psk{i}")
                       for i in range(len(chains))]
                for k in range(KT_HID):
                    for i, (g, cc) in enumerate(chains):
                        nc.tensor.matmul(
                            pss[i],
                            wk_sb[:, k, g * P:(g + 1) * P],
                            xt_sb[:, k, cc * T:(cc + 1) * T],
                            start=(k == 0), stop=(k == KT_HID - 1))
                for i, (g, cc) in enumerate(chains):
                    _rope(nc, kt_sb[:, g, cc * T:(cc + 1) * T], pss[i],
                          cosk_sb, sink_sb, ropet)

            def v_wave():
                pss = [psB.tile([P, T], FP32, tag="projps", name=f"psv{i}")
                       for i in range(NTOK_T)]
                for k in range(KT_HID):
                    for tt in range(NTOK_T):
                        nc.tensor.matmul(
                            pss[tt][:, :NKV * D],
                            xt_sb[:, k, tt * P:(tt + 1) * P],
                            wv_sb[:, k, :],
                            start=(k == 0), stop=(k == KT_HID - 1))
                for tt in range(NTOK_T):
                    # alternate copy engines so the drain is not ACT-serial
                    if tt % 2 == 0:
                        nc.scalar.copy(v_sb[:, tt, :], pss[tt][:, :NKV * D])
                    else:
                        nc.vector.tensor_copy(v_sb[:, tt, :],
                                              pss[tt][:, :NKV * D])

            q_wave(0)
            k_wave()
            v_wave()
            q_wave(1)

        # ---- phase C: attention, head pairs, one-pair software pipeline ----
        otpool = ctx.enter_context(tc.tile_pool(name="otpool", bufs=1))
        ot_sb = otpool.tile([P, NH, TOK], BF16, tag="ot")      # [d, h, tok]
        wopool = ctx.enter_context(tc.tile_pool(name="wopool", bufs=1))
        wo_sb = wopool.tile([P, KT_HID, HID], BF16, tag="wo")
        nc.sync.dma_start(out=wo_sb, in_=wo.rearrange("(k p) n -> p k n", p=P))
        # normalization state lives past phase C (batch 1's normalization
        # overlaps batch 0's output projection)
        normp = ctx.enter_context(tc.tile_pool(name="normp", bufs=8))
        sums = [normp.tile([NH, T], FP32, tag=f"sums{b}", name=f"sums{b}")
                for b in range(BL)]
        psR = ctx.enter_context(
            tc.tile_pool(name="psR", bufs=2, space=bass.MemorySpace.PSUM))

        with ExitStack() as phase2:
            ptpool = phase2.enter_context(tc.tile_pool(name="ptpool", bufs=2))
            # PSUM budget is exactly 8 banks: ST pair tile 2 (bufs=1 is
            # enough -- by emission order the PE runs CSOT(p-1) between
            # ST(p) and ST(p+1), so exp(p) has long drained the tile) +
            # o_ps 2 + cs 2 + rank-1 broadcast 2 (outer pool).
            psS = phase2.enter_context(
                tc.tile_pool(name="psS", bufs=1, space=bass.MemorySpace.PSUM))
            psO = phase2.enter_context(
                tc.tile_pool(name="psO", bufs=2, space=bass.MemorySpace.PSUM))
            psC = phase2.enter_context(
                tc.tile_pool(name="psC", bufs=2, space=bass.MemorySpace.PSUM))

            mask_b = maskt_sb[:, None, :].to_broadcast((P, GROUPS, P))

            def emit_st(b, g):
                """ST matmuls + exp + mask for one head pair; returns pt."""
                st = psS.tile([P, GROUPS, T], FP32, tag="sps")
                pt = ptpool.tile([P, GROUPS, NTQ, T], BF16, tag="pt")
                for j in range(NTQ):
                    lo = j * P
                    for hh in range(GROUPS):
                        h = GROUPS * g + hh
                        nc.tensor.matmul(
                            st[:, hh, lo:T],
                            kt_sb[:, g, b * T + lo: b * T + lo + P],
                            qt_sb[:, h, b * T + lo: (b + 1) * T],
                            start=True, stop=True)
                    # one exp per (pair, j): 3D AP spanning both psum banks
                    # (no row-max: logits are O(1) by construction)
                    nc.scalar.activation(
                        out=pt[:, :, j, lo:T], in_=st[:, :, lo:T],
                        func=mybir.ActivationFunctionType.Exp,
                        bias=0.0, scale=1.0)
                    # causal mask on the diagonal block, both heads in one
                    # op (mask operand broadcast along the head axis), on
                    # the otherwise-idle gpsimd
                    nc.gpsimd.tensor_mul(
                        pt[:, :, j, lo:lo + P], pt[:, :, j, lo:lo + P],
                        mask_b)
                return pt

            def emit_csot_mm(b, g, pt):
                """colsum + OT accumulation; denominator stashed for the
                batched normalization."""
                for hh in range(GROUPS):
                    h = GROUPS * g + hh
                    o_ps = psO.tile([P, T], FP32, tag="ops")
                    cs_ps = psC.tile([1, T], FP32, tag="cps")
                    # colsum matmuls first so the denominator stash launches
                    # before the OT matmuls run
                    for j in range(NTQ):
                        lo = j * P
                        nc.tensor.matmul(
                            cs_ps[:, lo:T] if j else cs_ps[:, :],
                            ones_bf[:, 0:1],
                            pt[:, hh, j, lo:T],
                            start=(j == 0), stop=(j == NTQ - 1),
                            skip_group_check=True)
                    # copy the [1,T] row out of PSUM (split across ACT/DVE)
                    # and DMA-stash it into partition h of sums[b] so the
                    # whole batch's reciprocal runs 8-lane-wide later
                    csrow = normp.tile([1, T], FP32, tag="csrow")
                    if hh == 0:
                        nc.scalar.copy(csrow, cs_ps)
                    else:
                        nc.vector.tensor_copy(csrow, cs_ps)
                    nc.sync.dma_start(out=sums[b][h:h + 1, :], in_=csrow)
                    for j in range(NTQ):
                        lo = j * P
                        nc.tensor.matmul(
                            o_ps[:, lo:T] if j else o_ps[:, :],
                            v_sb[:, b * NTQ + j, g * D:(g + 1) * D],
                            pt[:, hh, j, lo:T],
                            start=(j == 0), stop=(j == NTQ - 1),
                            skip_group_check=True)
                    # unnormalized OT out of PSUM immediately (frees banks;
                    # the normalization multiply lands later, in-place)
                    nc.vector.tensor_copy(
                        ot_sb[:, h, b * T:(b + 1) * T], o_ps)

            def emit_batch_recip(b):
                """one 8-lane-wide reciprocal for the whole batch, then
                extract each head's row back to partition 0 for the rank-1
                broadcast. Runs while the next batch (or the output
                projection) keeps the PE busy."""
                rinv_f = normp.tile([NH, T], FP32, tag="rinvf",
                                    name=f"rinvf{b}")
                nc.vector.reciprocal_approx_fast(out=rinv_f, in_=sums[b])
                rinv_bf = normp.tile([NH, T], BF16, tag="rinvbf",
                                     name=f"rinvbf{b}")
                nc.vector.tensor_copy(rinv_bf, rinv_f)
                rrows = []
                for h in range(NH):
                    rrow = normp.tile([1, T], BF16, tag="rrow",
                                      name=f"rrow{b}_{h}")
                    nc.sync.dma_start(out=rrow, in_=rinv_bf[h:h + 1, :])
                    rrows.append(rrow)
                return rrows

            def emit_norm_heads(b, rrows, heads):
                """rank-1 broadcast + in-place multiply for a few heads;
                chunks are interspersed between matmul blocks so the PE
                never waits on the (cheap but latent) reciprocal chain."""
                for h in heads:
                    rb_ps = psR.tile([P, T], FP32, tag="rbps")
                    nc.tensor.matmul(rb_ps, ones_bf[0:1, :], rrows[h],
                                     start=True, stop=True,
                                     skip_group_check=True)
                    nc.vector.tensor_mul(
                        ot_sb[:, h, b * T:(b + 1) * T],
                        ot_sb[:, h, b * T:(b + 1) * T], rb_ps)

            # software-pipelined schedule: csot lags ST by one pair; batch
            # 0's normalization hides under batch 1's attention; batch 1's
            # hides under batch 0's output projection (emitted in phase D)
            pts = {}
            pairs = [(b, g) for b in range(BL) for g in range(NKV)]
            for b, g in pairs:
                pts[(b, g)] = None
            pts[(0, 0)] = emit_st(0, 0)
            for i in range(1, 4):
                pts[(0, i)] = emit_st(0, i)
                emit_csot_mm(0, i - 1, pts[(0, i - 1)])
            pts[(1, 0)] = emit_st(1, 0)
            emit_csot_mm(0, 3, pts[(0, 3)])
            pts[(1, 1)] = emit_st(1, 1)
            emit_csot_mm(1, 0, pts[(1, 0)])
            rrows0 = emit_batch_recip(0)
            emit_norm_heads(0, rrows0, [0, 1])
            pts[(1, 2)] = emit_st(1, 2)
            emit_csot_mm(1, 1, pts[(1, 1)])
            emit_norm_heads(0, rrows0, [2, 3])
            pts[(1, 3)] = emit_st(1, 3)
            emit_csot_mm(1, 2, pts[(1, 2)])
            emit_norm_heads(0, rrows0, [4, 5])
            emit_csot_mm(1, 3, pts[(1, 3)])
            emit_norm_heads(0, rrows0, [6, 7])
            rrows1 = emit_batch_recip(1)

        # ---- phase D: output projection (b0 tiles overlap b1's norm) ----
        with ExitStack() as phase3:
            opool = phase3.enter_context(tc.tile_pool(name="opool", bufs=3))
            psD = phase3.enter_context(
                tc.tile_pool(name="psD", bufs=3, space=bass.MemorySpace.PSUM))
            NCH = HID // T  # 2 chunks of 512

            def emit_out_tile(tt):
                o_tile = opool.tile([P, HID], BF16, tag="o")
                # interleave both output chunks k-major: consecutive matmul
                # pairs share the stationary operand OT[:,k,tt-block]
                ps0 = psD.tile([P, T], FP32, tag="dps0")
                ps1 = psD.tile([P, T], FP32, tag="dps1")
                pss = [ps0, ps1]
                for k in range(KT_HID):
                    for cchunk in range(NCH):
                        nc.tensor.matmul(
                            pss[cchunk],
                            ot_sb[:, k, tt * P:(tt + 1) * P],
                            wo_sb[:, k, cchunk * T:(cchunk + 1) * T],
                            start=(k == 0), stop=(k == KT_HID - 1))
                # alternate engines so the copies run in parallel
                nc.vector.tensor_copy(o_tile[:, 0:T], pss[0])
                nc.scalar.copy(o_tile[:, T:HID], pss[1])
                eng = nc.sync if tt % 2 == 0 else nc.scalar
                eng.dma_start(out=out[tt * P:(tt + 1) * P, :], in_=o_tile)

            emit_out_tile(0)
            emit_norm_heads(1, rrows1, [0, 1])
            emit_out_tile(1)
            emit_norm_heads(1, rrows1, [2, 3])
            emit_out_tile(2)
            emit_norm_heads(1, rrows1, [4, 5])
            emit_out_tile(3)
            emit_norm_heads(1, rrows1, [6, 7])
            for tt in range(4, NTOK_T):
                emit_out_tile(tt)


_COMPILED = None


def _get_compiled():
    global _COMPILED
    if _COMPILED is None:
        nc = bacc.Bacc("TRN2", target_bir_lowering=False, debug=False)
        _build(nc)
        nc.compile()
        _COMPILED = nc
    return _COMPILED


def kernel(hidden_states, Wq, Wk, Wv, Wo, _trace=False, _trace_kwargs=None):
    hs = np.asarray(hidden_states, dtype=np.float32).astype(BF)
    wq = np.ascontiguousarray(np.asarray(Wq, dtype=np.float32).astype(BF))
    wk = np.ascontiguousarray(np.asarray(Wk, dtype=np.float32).astype(BF))
    wv = np.ascontiguousarray(np.asarray(Wv, dtype=np.float32).astype(BF))
    wo = np.ascontiguousarray(np.asarray(Wo, dtype=np.float32).astype(BF))
    consts = _host_consts()
    nc = _get_compiled()
    in_maps = []
    for c in range(NCORES):
        # ship X pre-transposed ([HID, TOK]) so the kernel's lhs/rhs layouts
        # need no on-chip transpose of X at all
        shard_t = np.ascontiguousarray(
            hs[BL * c: BL * (c + 1)].reshape(TOK, HID).T)
        in_maps.append({"hidden_t": shard_t, "Wq": wq, "Wk": wk, "Wv": wv,
                        "Wo": wo, **consts})
    res = run_bass_kernel_spmd(
        nc, in_maps, list(range(NCORES)), trace=_trace,
        **(_trace_kwargs or {}))
    outs = [r["out"].astype(np.float32).reshape(BL, T, HID)
            for r in res.results]
    full = np.concatenate(outs, axis=0)
    if _trace:
        return full, res
    return full


# revision 23
# speedup vs baseline: 2.0164x; 1.1488x over previous
"""GQA attention kernel for Trainium2, data-parallel over batch on 8 NeuronCores.

Per-core problem (2 of 16 batches): X [1024tok, 1024] -> QKV proj -> RoPE ->
causal GQA attention (8 q heads, 4 kv heads, D=128) -> out proj [1024, 1024].

Layout strategy: everything stays in "feature-on-partition" transposed form,
and attention scores are computed TRANSPOSED (ST[tk,tq]) so that exp(ST) is
already the P.T the PV matmul needs -- no transposes of P at all. Matmul
operands are bf16 (fp32 PSUM accumulate).

Schedule (all per core):
  XT[hid,tok]   = host-pretransposed X                     (DRAM -> SBUF)
  QT[dq,tok]    = Wq.T @ XT   k-outer waves of 8 chains  + RoPE
  KT[dkv,tok]   = Wk.T @ XT   + RoPE
  V [tok,dkv]   = X @ Wv      (lhsT = XT, rhs = Wv)
  attention per (batch, kv-group) head PAIR, software-pipelined one pair deep:
    ST[tk, 2, tq] = KT_j.T @ QT_h  per 128-row tk block, causal col range,
                    both heads of the group into one 2-bank psum tile
    PT            = exp(ST)  one ACT op per (pair, j) via 3D AP
    mask          = one DVE mul per (pair, j) over both heads' diag blocks
                    (stride-0 broadcast mask operand)
    colsum[1,tq] += ones[128,1].T @ PT_j    (PE)
    OT[d,tq]     += V_j.T @ PT_j            (PE accumulate over j)
    norm per head, pipelined entirely off the PE:
      cs row -> scratch-DRAM roundtrip transpose -> WIDE reciprocal [128,4]
      -> roundtrip back -> DMA partition-broadcast [128,T] -> in-place
      SBUF multiply of the already-copied-out OT
  Out[tok,hid]  = OT.T @ Wo   -> bf16 store (host casts back to fp32)
RoPE scale 1/sqrt(D) is folded into the Q cos/sin host constants.
"""

import numpy as np
import ml_dtypes
from contextlib import ExitStack

import concourse.bass as bass
import concourse.tile as tile
from concourse import bacc, mybir
from concourse.bass_utils import run_bass_kernel_spmd

B, T, HID = 16, 512, 1024
NH, NKV, D = 8, 4, 128
THETA = 10000.0
NCORES = 8
BL = B // NCORES          # local batches per core
TOK = BL * T              # local tokens
P = 128
KT_HID = HID // P         # 8 contraction tiles over hidden
NTQ = T // P              # 4 tk/tq tiles per sequence
GROUPS = NH // NKV        # 2 q heads per kv head
NTOK_T = TOK // P         # 8 token tiles per core
FP32 = mybir.dt.float32
BF16 = mybir.dt.bfloat16
BF = ml_dtypes.bfloat16


def _host_consts():
    inv_freq = 1.0 / (THETA ** (np.arange(0, D, 2, dtype=np.float64) / D))
    freqs = np.outer(np.arange(T, dtype=np.float64), inv_freq)    # [T, 64]
    emb = np.concatenate([freqs, freqs], axis=-1)                 # [T, 128]
    cos = np.cos(emb).T                                           # [128, T]
    sin = np.sin(emb).T
    scale = 1.0 / np.sqrt(D)
    # rotate_half sign folded into sin: out = q*cos + qswap*sin_signed where
    # qswap is q with its partition halves swapped
    sin_signed = np.concatenate([-sin[:D // 2], sin[D // 2:]], axis=0)
    # transposed-S diagonal-block multiplicative mask: rows tk, cols tq;
    # valid iff tq >= tk
    mask_t = np.triu(np.ones((P, P), np.float32)).astype(BF)
    return {
        "cos_q": (cos * scale).astype(BF),
        "sin_q": (sin_signed * scale).astype(BF),
        "cos_k": cos.astype(BF),
        "sin_k": sin_signed.astype(BF),
        "mask_t": mask_t,
    }


def _rope(nc, out_sl, psum, cos_sb, sin_sb, tmp_pool):
    """out = q * cos + rotate_half(q) * sin for q = psum, all [128, T] APs.

    One ACT copy moves psum -> bf16 SBUF (single slow PSUM read), then the
    arithmetic runs in the DVE's fast bf16-SBUF mode.
    """
    H = D // 2
    qraw = tmp_pool.tile([P, T], BF16, tag="rope_raw")
    nc.scalar.copy(qraw, psum)
    # partition-half swap of rotate_half runs on the DMA engine (compute
    # engines cannot shift partitions between SBUF operands); the sign of
    # rotate_half is folded into the host sin constant
    qswap = tmp_pool.tile([P, T], BF16, tag="rope_swap")
    nc.sync.dma_start(out=qswap[0:H], in_=qraw[H:P])
    nc.sync.dma_start(out=qswap[H:P], in_=qraw[0:H])
    tmp = tmp_pool.tile([P, T], BF16, tag="rope_tmp")
    nc.gpsimd.tensor_mul(tmp, qswap, sin_sb)
    nc.vector.tensor_mul(out_sl, qraw, cos_sb)
    nc.vector.tensor_add(out_sl, out_sl, tmp)


def _build(nc):
    # hidden arrives pre-transposed from the host: [HID, TOK]
    hid_t = nc.dram_tensor("hidden_t", [HID, TOK], BF16,
                           kind="ExternalInput").ap()
    wq = nc.dram_tensor("Wq", [HID, NH * D], BF16, kind="ExternalInput").ap()
    wk = nc.dram_tensor("Wk", [HID, NKV * D], BF16, kind="ExternalInput").ap()
    wv = nc.dram_tensor("Wv", [HID, NKV * D], BF16, kind="ExternalInput").ap()
    wo = nc.dram_tensor("Wo", [NH * D, HID], BF16, kind="ExternalInput").ap()
    cos_q = nc.dram_tensor("cos_q", [P, T], BF16, kind="ExternalInput").ap()
    sin_q = nc.dram_tensor("sin_q", [P, T], BF16, kind="ExternalInput").ap()
    cos_k = nc.dram_tensor("cos_k", [P, T], BF16, kind="ExternalInput").ap()
    sin_k = nc.dram_tensor("sin_k", [P, T], BF16, kind="ExternalInput").ap()
    mask_t = nc.dram_tensor("mask_t", [P, P], BF16, kind="ExternalInput").ap()
    out = nc.dram_tensor("out", [TOK, HID], BF16, kind="ExternalOutput").ap()

    with tile.TileContext(nc) as tc, ExitStack() as ctx:
        # ---- pools with cross-phase lifetimes ----
        consts = ctx.enter_context(tc.tile_pool(name="consts", bufs=1))

        cosq_sb = consts.tile([P, T], BF16, tag="cq")
        sinq_sb = consts.tile([P, T], BF16, tag="sq")
        cosk_sb = consts.tile([P, T], BF16, tag="ck")
        sink_sb = consts.tile([P, T], BF16, tag="sk")
        maskt_sb = consts.tile([P, P], BF16, tag="maskt")
        ones_bf = consts.tile([P, P], BF16, tag="ones")
        warm_rhs = consts.tile([P, 256], BF16, tag="warm")
        # memsets on the (otherwise idle at startup) gpsimd engine so the
        # PE warmup can begin as early as possible
        nc.gpsimd.memset(ones_bf, 1.0)
        nc.gpsimd.memset(warm_rhs, 0.0)

        qkvpool = ctx.enter_context(tc.tile_pool(name="qkv", bufs=1))
        qt_sb = qkvpool.tile([P, NH, TOK], BF16, tag="qt")     # [d, h, tok]
        kt_sb = qkvpool.tile([P, NKV, TOK], BF16, tag="kt")    # [d, g, tok]
        v_sb = qkvpool.tile([P, NTOK_T, NKV * D], BF16, tag="v")  # [tok,tt,dkv]

        # ---- phase A+B: loads + QKV projections (k-outer waves) ----
        with ExitStack() as phase1:
            wpool = phase1.enter_context(tc.tile_pool(name="wpool", bufs=1))
            xtp = phase1.enter_context(tc.tile_pool(name="xtp", bufs=1))
            ropet = phase1.enter_context(tc.tile_pool(name="ropet", bufs=4))
            psB = phase1.enter_context(
                tc.tile_pool(name="psB", bufs=8, space=bass.MemorySpace.PSUM))

            wq_sb = wpool.tile([P, KT_HID, NH * D], BF16, tag="wq")
            wk_sb = wpool.tile([P, KT_HID, NKV * D], BF16, tag="wk")
            wv_sb = wpool.tile([P, KT_HID, NKV * D], BF16, tag="wv")
            xt_sb = xtp.tile([P, KT_HID, TOK], BF16, tag="xt")  # [hid, k, tok]
            wq_r = wq.rearrange("(k p) n -> p k n", p=P)
            wk_r = wk.rearrange("(k p) n -> p k n", p=P)
            wv_r = wv.rearrange("(k p) n -> p k n", p=P)
            hid_r = hid_t.rearrange("(k p) t -> p k t", p=P)
            # load order follows consumption order: the Q projection streams
            # k-chunk by k-chunk, so (xt[k], wq[k]) pairs go first, split
            # across the two HARDWARE DGE rings (sync + scalar; the gpsimd
            # ring is software DGE executed on the Q7 cores -- never use it
            # for bulk). RoPE consts next; then wk/wv/wo.
            for k in range(KT_HID):
                nc.sync.dma_start(out=xt_sb[:, k, :], in_=hid_r[:, k, :])
                nc.scalar.dma_start(out=wq_sb[:, k, :], in_=wq_r[:, k, :])
            nc.sync.dma_start(out=cosq_sb, in_=cos_q)
            nc.sync.dma_start(out=sinq_sb, in_=sin_q)
            nc.scalar.dma_start(out=cosk_sb, in_=cos_k)
            nc.scalar.dma_start(out=sink_sb, in_=sin_k)
            nc.scalar.dma_start(out=maskt_sb, in_=mask_t)
            for k in range(KT_HID):
                nc.sync.dma_start(out=wk_sb[:, k, :], in_=wk_r[:, k, :])
                nc.scalar.dma_start(out=wv_sb[:, k, :], in_=wv_r[:, k, :])

            # PE warmup: ~3.5us of dependency-light matmuls ahead of the
            # first projection so the HAM clock-gate releases (1.2 ->
            # 2.4 GHz) while the input DMAs are still in flight
            wps = psB.tile([P, T], FP32, tag="projps")
            for _ in range(16):
                nc.tensor.matmul(wps[:, 0:256], ones_bf, warm_rhs,
                                 start=True, stop=True, skip_group_check=True)

            def q_wave(c):
                # k-outer: the first matmuls need only (xt[0], wq[0]), so
                # the PE starts when the first 512KB lands, not after the
                # whole load; DMA delivery and PE consumption are balanced.
                pss = [psB.tile([P, T], FP32, tag="projps",
                                name=f"psq{c}_{i}") for i in range(NH)]
                for k in range(KT_HID):
                    for h in range(NH):
                        nc.tensor.matmul(
                            pss[h],
                            wq_sb[:, k, h * P:(h + 1) * P],
                            xt_sb[:, k, c * T:(c + 1) * T],
                            start=(k == 0), stop=(k == KT_HID - 1))
                for h in range(NH):
                    _rope(nc, qt_sb[:, h, c * T:(c + 1) * T], pss[h],
                          cosq_sb, sinq_sb, ropet)

            def k_wave():
                chains = [(g, cc) for g in range(NKV) for cc in range(BL)]
                pss = [psB.tile([P, T], FP32, tag="projps", name=f"psk{i}")
                       for i in range(len(chains))]
                for k in range(KT_HID):
                    for i, (g, cc) in enumerate(chains):
                        nc.tensor.matmul(
                            pss[i],
                            wk_sb[:, k, g * P:(g + 1) * P],
                            xt_sb[:, k, cc * T:(cc + 1) * T],
                            start=(k == 0), stop=(k == KT_HID - 1))
                for i, (g, cc) in enumerate(chains):
                    _rope(nc, kt_sb[:, g, cc * T:(cc + 1) * T], pss[i],
                          cosk_sb, sink_sb, ropet)

            def v_wave():
                pss = [psB.tile([P, T], FP32, tag="projps", name=f"psv{i}")
                       for i in range(NTOK_T)]
                for k in range(KT_HID):
                    for tt in range(NTOK_T):
                        nc.tensor.matmul(
                            pss[tt][:, :NKV * D],
                            xt_sb[:, k, tt * P:(tt + 1) * P],
                            wv_sb[:, k, :],
                            start=(k == 0), stop=(k == KT_HID - 1))
                for tt in range(NTOK_T):
                    # alternate copy engines so the drain is not ACT-serial
                    if tt % 2 == 0:
                        nc.scalar.copy(v_sb[:, tt, :], pss[tt][:, :NKV * D])
                    else:
                        nc.vector.tensor_copy(v_sb[:, tt, :],
                                              pss[tt][:, :NKV * D])

            # V last: the attention phase's first psum allocations alias
            # the last wave's banks, and V's copies drain fastest (a RoPE
            # tail would stall the first ST matmuls ~7us)
            q_wave(0)
            q_wave(1)
            k_wave()
            v_wave()

        # ---- phase C: attention, head pairs, one-pair software pipeline ----
        otpool = ctx.enter_context(tc.tile_pool(name="otpool", bufs=1))
        ot_sb = otpool.tile([P, NH, TOK], BF16, tag="ot")      # [d, h, tok]
        wopool = ctx.enter_context(tc.tile_pool(name="wopool", bufs=1))
        wo_sb = wopool.tile([P, KT_HID, HID], BF16, tag="wo")
        nc.sync.dma_start(out=wo_sb, in_=wo.rearrange("(k p) n -> p k n", p=P))
        # normalization state lives past phase C (batch 1's normalization
        # overlaps batch 0's output projection)
        normp = ctx.enter_context(tc.tile_pool(name="normp", bufs=8))
        sums = [normp.tile([NH, T], FP32, tag=f"sums{b}", name=f"sums{b}")
                for b in range(BL)]
        psR = ctx.enter_context(
            tc.tile_pool(name="psR", bufs=2, space=bass.MemorySpace.PSUM))

        with ExitStack() as phase2:
            ptpool = phase2.enter_context(tc.tile_pool(name="ptpool", bufs=2))
            # PSUM budget is exactly 8 banks: ST pair tiles 2x2 (per-j
            # tiles, double-buffered, so ST(j+1) never waits for exp(j) to
            # drain an overlapping region) + o_ps 1 + cs 1 (the cs/OT/copy
            # interleave within a pair covers the single-buffer WARs) +
            # rank-1 broadcast 2 (outer pool).
            psS = phase2.enter_context(
                tc.tile_pool(name="psS", bufs=2, space=bass.MemorySpace.PSUM))
            psO = phase2.enter_context(
                tc.tile_pool(name="psO", bufs=1, space=bass.MemorySpace.PSUM))
            psC = phase2.enter_context(
                tc.tile_pool(name="psC", bufs=1, space=bass.MemorySpace.PSUM))

            mask_b = maskt_sb[:, None, :].to_broadcast((P, GROUPS, P))

            def emit_st(b, g):
                """ST matmuls + exp + mask for one head pair; returns pt."""
                pt = ptpool.tile([P, GROUPS, NTQ, T], BF16, tag="pt")
                for j in range(NTQ):
                    lo = j * P
                    st = psS.tile([P, GROUPS, T], FP32, tag="sps")
                    for hh in range(GROUPS):
                        h = GROUPS * g + hh
                        nc.tensor.matmul(
                            st[:, hh, lo:T],
                            kt_sb[:, g, b * T + lo: b * T + lo + P],
                            qt_sb[:, h, b * T + lo: (b + 1) * T],
                            start=True, stop=True)
                    # one exp per (pair, j): 3D AP spanning both psum banks
                    # (no row-max: logits are O(1) by construction)
                    nc.scalar.activation(
                        out=pt[:, :, j, lo:T], in_=st[:, :, lo:T],
                        func=mybir.ActivationFunctionType.Exp,
                        bias=0.0, scale=1.0)
                    # causal mask on the diagonal block, both heads in one
                    # op (mask operand broadcast along the head axis), on
                    # the otherwise-idle gpsimd
                    nc.gpsimd.tensor_mul(
                        pt[:, :, j, lo:lo + P], pt[:, :, j, lo:lo + P],
                        mask_b)
                return pt

            def emit_csot_mm(b, g, pt):
                """colsum + OT accumulation; denominator stashed for the
                batched normalization."""
                for hh in range(GROUPS):
                    h = GROUPS * g + hh
                    o_ps = psO.tile([P, T], FP32, tag="ops")
                    cs_ps = psC.tile([1, T], FP32, tag="cps")
                    # colsum matmuls first so the denominator stash launches
                    # before the OT matmuls run
                    for j in range(NTQ):
                        lo = j * P
                        nc.tensor.matmul(
                            cs_ps[:, lo:T] if j else cs_ps[:, :],
                            ones_bf[:, 0:1],
                            pt[:, hh, j, lo:T],
                            start=(j == 0), stop=(j == NTQ - 1),
                            skip_group_check=True)
                    # copy the [1,T] row out of PSUM (split across ACT/DVE)
                    # and DMA-stash it into partition h of sums[b] so the
                    # whole batch's reciprocal runs 8-lane-wide later
                    csrow = normp.tile([1, T], FP32, tag="csrow")
                    if hh == 0:
                        nc.scalar.copy(csrow, cs_ps)
                    else:
                        nc.vector.tensor_copy(csrow, cs_ps)
                    nc.sync.dma_start(out=sums[b][h:h + 1, :], in_=csrow)
                    for j in range(NTQ):
                        lo = j * P
                        nc.tensor.matmul(
                            o_ps[:, lo:T] if j else o_ps[:, :],
                            v_sb[:, b * NTQ + j, g * D:(g + 1) * D],
                            pt[:, hh, j, lo:T],
                            start=(j == 0), stop=(j == NTQ - 1),
                            skip_group_check=True)
                    # unnormalized OT out of PSUM immediately (frees banks;
                    # the normalization multiply lands later, in-place)
                    nc.vector.tensor_copy(
                        ot_sb[:, h, b * T:(b + 1) * T], o_ps)

            def emit_batch_recip(b):
                """one 8-lane-wide reciprocal for the whole batch, then
                extract each head's row back to partition 0 for the rank-1
                broadcast. Runs while the next batch (or the output
                projection) keeps the PE busy."""
                rinv_f = normp.tile([NH, T], FP32, tag="rinvf",
                                    name=f"rinvf{b}")
                nc.vector.reciprocal_approx_fast(out=rinv_f, in_=sums[b])
                rinv_bf = normp.tile([NH, T], BF16, tag="rinvbf",
                                     name=f"rinvbf{b}")
                nc.vector.tensor_copy(rinv_bf, rinv_f)
                rrows = []
                for h in range(NH):
                    rrow = normp.tile([1, T], BF16, tag="rrow",
                                      name=f"rrow{b}_{h}")
                    nc.sync.dma_start(out=rrow, in_=rinv_bf[h:h + 1, :])
                    rrows.append(rrow)
                return rrows

            def emit_norm_heads(b, rrows, heads):
                """rank-1 broadcast + in-place multiply for a few heads;
                chunks are interspersed between matmul blocks so the PE
                never waits on the (cheap but latent) reciprocal chain."""
                for h in heads:
                    rb_ps = psR.tile([P, T], FP32, tag="rbps")
                    nc.tensor.matmul(rb_ps, ones_bf[0:1, :], rrows[h],
                                     start=True, stop=True,
                                     skip_group_check=True)
                    nc.vector.tensor_mul(
                        ot_sb[:, h, b * T:(b + 1) * T],
                        ot_sb[:, h, b * T:(b + 1) * T], rb_ps)

            # software-pipelined schedule: csot lags ST by one pair; batch
            # 0's normalization hides under batch 1's attention; batch 1's
            # hides under batch 0's output projection (emitted in phase D)
            pts = {}
            pairs = [(b, g) for b in range(BL) for g in range(NKV)]
            for b, g in pairs:
                pts[(b, g)] = None
            pts[(0, 0)] = emit_st(0, 0)
            for i in range(1, 4):
                pts[(0, i)] = emit_st(0, i)
                emit_csot_mm(0, i - 1, pts[(0, i - 1)])
            pts[(1, 0)] = emit_st(1, 0)
            emit_csot_mm(0, 3, pts[(0, 3)])
            pts[(1, 1)] = emit_st(1, 1)
            emit_csot_mm(1, 0, pts[(1, 0)])
            rrows0 = emit_batch_recip(0)
            emit_norm_heads(0, rrows0, [0, 1])
            pts[(1, 2)] = emit_st(1, 2)
            emit_csot_mm(1, 1, pts[(1, 1)])
            emit_norm_heads(0, rrows0, [2, 3])
            pts[(1, 3)] = emit_st(1, 3)
            emit_csot_mm(1, 2, pts[(1, 2)])
            emit_norm_heads(0, rrows0, [4, 5])
            emit_csot_mm(1, 3, pts[(1, 3)])
            emit_norm_heads(0, rrows0, [6, 7])
            rrows1 = emit_batch_recip(1)

        # ---- phase D: output projection (b0 tiles overlap b1's norm) ----
        with ExitStack() as phase3:
            opool = phase3.enter_context(tc.tile_pool(name="opool", bufs=3))
            psD = phase3.enter_context(
                tc.tile_pool(name="psD", bufs=3, space=bass.MemorySpace.PSUM))
            NCH = HID // T  # 2 chunks of 512

            def emit_out_tile(tt):
                o_tile = opool.tile([P, HID], BF16, tag="o")
                # interleave both output chunks k-major: consecutive matmul
                # pairs share the stationary operand OT[:,k,tt-block]
                ps0 = psD.tile([P, T], FP32, tag="dps0")
                ps1 = psD.tile([P, T], FP32, tag="dps1")
                pss = [ps0, ps1]
                for k in range(KT_HID):
                    for cchunk in range(NCH):
                        nc.tensor.matmul(
                            pss[cchunk],
                            ot_sb[:, k, tt * P:(tt + 1) * P],
                            wo_sb[:, k, cchunk * T:(cchunk + 1) * T],
                            start=(k == 0), stop=(k == KT_HID - 1))
                # alternate engines so the copies run in parallel
                nc.vector.tensor_copy(o_tile[:, 0:T], pss[0])
                nc.scalar.copy(o_tile[:, T:HID], pss[1])
                eng = nc.sync if tt % 2 == 0 else nc.scalar
                eng.dma_start(out=out[tt * P:(tt + 1) * P, :], in_=o_tile)

            emit_out_tile(0)
            emit_norm_heads(1, rrows1, [0, 1])
            emit_out_tile(1)
            emit_norm_heads(1, rrows1, [2, 3])
            emit_out_tile(2)
            emit_norm_heads(1, rrows1, [4, 5])
            emit_out_tile(3)
            emit_norm_heads(1, rrows1, [6, 7])
            for tt in range(4, NTOK_T):
                emit_out_tile(tt)


_COMPILED = None


def _get_compiled():
    global _COMPILED
    if _COMPILED is None:
        nc = bacc.Bacc("TRN2", target_bir_lowering=False, debug=False)
        _build(nc)
        nc.compile()
        _COMPILED = nc
    return _COMPILED


def kernel(hidden_states, Wq, Wk, Wv, Wo, _trace=False, _trace_kwargs=None):
    hs = np.asarray(hidden_states, dtype=np.float32).astype(BF)
    wq = np.ascontiguousarray(np.asarray(Wq, dtype=np.float32).astype(BF))
    wk = np.ascontiguousarray(np.asarray(Wk, dtype=np.float32).astype(BF))
    wv = np.ascontiguousarray(np.asarray(Wv, dtype=np.float32).astype(BF))
    wo = np.ascontiguousarray(np.asarray(Wo, dtype=np.float32).astype(BF))
    consts = _host_consts()
    nc = _get_compiled()
    in_maps = []
    for c in range(NCORES):
        # ship X pre-transposed ([HID, TOK]) so the kernel's lhs/rhs layouts
        # need no on-chip transpose of X at all
        shard_t = np.ascontiguousarray(
            hs[BL * c: BL * (c + 1)].reshape(TOK, HID).T)
        in_maps.append({"hidden_t": shard_t, "Wq": wq, "Wk": wk, "Wv": wv,
                        "Wo": wo, **consts})
    res = run_bass_kernel_spmd(
        nc, in_maps, list(range(NCORES)), trace=_trace,
        **(_trace_kwargs or {}))
    outs = [r["out"].astype(np.float32).reshape(BL, T, HID)
            for r in res.results]
    full = np.concatenate(outs, axis=0)
    if _trace:
        return full, res
    return full


# revision 26
# speedup vs baseline: 2.0735x; 1.0283x over previous
"""GQA attention kernel for Trainium2, data-parallel over batch on 8 NeuronCores.

Per-core problem (2 of 16 batches): X [1024tok, 1024] -> QKV proj -> RoPE ->
causal GQA attention (8 q heads, 4 kv heads, D=128) -> out proj [1024, 1024].

Layout strategy: everything stays in "feature-on-partition" transposed form,
and attention scores are computed TRANSPOSED (ST[tk,tq]) so that exp(ST) is
already the P.T the PV matmul needs -- no transposes of P at all. Matmul
operands are bf16 (fp32 PSUM accumulate).

Schedule (all per core):
  XT[hid,tok]   = host-pretransposed X                     (DRAM -> SBUF)
  QT[dq,tok]    = Wq.T @ XT   k-outer waves of 8 chains  + RoPE
  KT[dkv,tok]   = Wk.T @ XT   + RoPE
  V [tok,dkv]   = X @ Wv      (lhsT = XT, rhs = Wv)
  attention per (batch, kv-group) head PAIR, software-pipelined one pair deep:
    ST[tk, 2, tq] = KT_j.T @ QT_h  per 128-row tk block, causal col range,
                    both heads of the group into one 2-bank psum tile
    PT            = exp(ST)  one ACT op per (pair, j) via 3D AP
    mask          = one DVE mul per (pair, j) over both heads' diag blocks
                    (stride-0 broadcast mask operand)
    colsum[1,tq] += ones[128,1].T @ PT_j    (PE)
    OT[d,tq]     += V_j.T @ PT_j            (PE accumulate over j)
    norm per head, pipelined entirely off the PE:
      cs row -> scratch-DRAM roundtrip transpose -> WIDE reciprocal [128,4]
      -> roundtrip back -> DMA partition-broadcast [128,T] -> in-place
      SBUF multiply of the already-copied-out OT
  Out[tok,hid]  = OT.T @ Wo   -> bf16 store (host casts back to fp32)
RoPE scale 1/sqrt(D) is folded into the Q cos/sin host constants.
"""

import numpy as np
import ml_dtypes
from contextlib import ExitStack

import concourse.bass as bass
import concourse.tile as tile
from concourse import bacc, mybir
from concourse.bass_utils import run_bass_kernel_spmd

B, T, HID = 16, 512, 1024
NH, NKV, D = 8, 4, 128
THETA = 10000.0
NCORES = 8
BL = B // NCORES          # local batches per core
TOK = BL * T              # local tokens
P = 128
KT_HID = HID // P         # 8 contraction tiles over hidden
NTQ = T // P              # 4 tk/tq tiles per sequence
GROUPS = NH // NKV        # 2 q heads per kv head
NTOK_T = TOK // P         # 8 token tiles per core
FP32 = mybir.dt.float32
BF16 = mybir.dt.bfloat16
BF = ml_dtypes.bfloat16


def _host_consts():
    inv_freq = 1.0 / (THETA ** (np.arange(0, D, 2, dtype=np.float64) / D))
    freqs = np.outer(np.arange(T, dtype=np.float64), inv_freq)    # [T, 64]
    emb = np.concatenate([freqs, freqs], axis=-1)                 # [T, 128]
    cos = np.cos(emb).T                                           # [128, T]
    sin = np.sin(emb).T
    scale = 1.0 / np.sqrt(D)
    # rotate_half sign folded into sin: out = q*cos + qswap*sin_signed where
    # qswap is q with its partition halves swapped
    sin_signed = np.concatenate([-sin[:D // 2], sin[D // 2:]], axis=0)
    # transposed-S diagonal-block multiplicative mask: rows tk, cols tq;
    # valid iff tq >= tk
    mask_t = np.triu(np.ones((P, P), np.float32)).astype(BF)
    return {
        "cos_q": (cos * scale).astype(BF),
        "sin_q": (sin_signed * scale).astype(BF),
        "cos_k": cos.astype(BF),
        "sin_k": sin_signed.astype(BF),
        "mask_t": mask_t,
    }


def _rope(nc, out_sl, psum, cos_sb, sin_sb, tmp_pool):
    """out = q * cos + rotate_half(q) * sin for q = psum, all [128, T] APs.

    One ACT copy moves psum -> bf16 SBUF (single slow PSUM read), then the
    arithmetic runs in the DVE's fast bf16-SBUF mode.
    """
    H = D // 2
    qraw = tmp_pool.tile([P, T], BF16, tag="rope_raw")
    nc.scalar.copy(qraw, psum)
    # partition-half swap of rotate_half runs on the DMA engine (compute
    # engines cannot shift partitions between SBUF operands); the sign of
    # rotate_half is folded into the host sin constant
    qswap = tmp_pool.tile([P, T], BF16, tag="rope_swap")
    nc.sync.dma_start(out=qswap[0:H], in_=qraw[H:P])
    nc.sync.dma_start(out=qswap[H:P], in_=qraw[0:H])
    tmp = tmp_pool.tile([P, T], BF16, tag="rope_tmp")
    nc.gpsimd.tensor_mul(tmp, qswap, sin_sb)
    nc.vector.tensor_mul(out_sl, qraw, cos_sb)
    nc.vector.tensor_add(out_sl, out_sl, tmp)


def _build(nc):
    # hidden arrives pre-transposed from the host: [HID, TOK]
    hid_t = nc.dram_tensor("hidden_t", [HID, TOK], BF16,
                           kind="ExternalInput").ap()
    wq = nc.dram_tensor("Wq", [HID, NH * D], BF16, kind="ExternalInput").ap()
    wk = nc.dram_tensor("Wk", [HID, NKV * D], BF16, kind="ExternalInput").ap()
    wv = nc.dram_tensor("Wv", [HID, NKV * D], BF16, kind="ExternalInput").ap()
    wo = nc.dram_tensor("Wo", [NH * D, HID], BF16, kind="ExternalInput").ap()
    cos_q = nc.dram_tensor("cos_q", [P, T], BF16, kind="ExternalInput").ap()
    sin_q = nc.dram_tensor("sin_q", [P, T], BF16, kind="ExternalInput").ap()
    cos_k = nc.dram_tensor("cos_k", [P, T], BF16, kind="ExternalInput").ap()
    sin_k = nc.dram_tensor("sin_k", [P, T], BF16, kind="ExternalInput").ap()
    mask_t = nc.dram_tensor("mask_t", [P, P], BF16, kind="ExternalInput").ap()
    out = nc.dram_tensor("out", [TOK, HID], BF16, kind="ExternalOutput").ap()

    with tile.TileContext(nc) as tc, ExitStack() as ctx:
        # ---- pools with cross-phase lifetimes ----
        consts = ctx.enter_context(tc.tile_pool(name="consts", bufs=1))

        cosq_sb = consts.tile([P, T], BF16, tag="cq")
        sinq_sb = consts.tile([P, T], BF16, tag="sq")
        cosk_sb = consts.tile([P, T], BF16, tag="ck")
        sink_sb = consts.tile([P, T], BF16, tag="sk")
        maskt_sb = consts.tile([P, P], BF16, tag="maskt")
        ones_bf = consts.tile([P, P], BF16, tag="ones")
        warm_rhs = consts.tile([P, 256], BF16, tag="warm")
        # memsets on the (otherwise idle at startup) gpsimd engine so the
        # PE warmup can begin as early as possible
        nc.gpsimd.memset(ones_bf, 1.0)
        nc.gpsimd.memset(warm_rhs, 0.0)

        qkvpool = ctx.enter_context(tc.tile_pool(name="qkv", bufs=1))
        qt_sb = qkvpool.tile([P, NH, TOK], BF16, tag="qt")     # [d, h, tok]
        kt_sb = qkvpool.tile([P, NKV, TOK], BF16, tag="kt")    # [d, g, tok]
        v_sb = qkvpool.tile([P, NTOK_T, NKV * D], BF16, tag="v")  # [tok,tt,dkv]

        # ---- phase A+B: loads + QKV projections (k-outer waves) ----
        with ExitStack() as phase1:
            wpool = phase1.enter_context(tc.tile_pool(name="wpool", bufs=1))
            xtp = phase1.enter_context(tc.tile_pool(name="xtp", bufs=1))
            ropet = phase1.enter_context(tc.tile_pool(name="ropet", bufs=4))
            psB = phase1.enter_context(
                tc.tile_pool(name="psB", bufs=8, space=bass.MemorySpace.PSUM))

            wq_sb = wpool.tile([P, KT_HID, NH * D], BF16, tag="wq")
            wk_sb = wpool.tile([P, KT_HID, NKV * D], BF16, tag="wk")
            wv_sb = wpool.tile([P, KT_HID, NKV * D], BF16, tag="wv")
            xt_sb = xtp.tile([P, KT_HID, TOK], BF16, tag="xt")  # [hid, k, tok]
            wq_r = wq.rearrange("(k p) n -> p k n", p=P)
            wk_r = wk.rearrange("(k p) n -> p k n", p=P)
            wv_r = wv.rearrange("(k p) n -> p k n", p=P)
            hid_r = hid_t.rearrange("(k p) t -> p k t", p=P)
            # load order follows consumption order: the Q projection streams
            # k-chunk by k-chunk, so (xt[k], wq[k]) pairs go first, split
            # across the two HARDWARE DGE rings (sync + scalar; the gpsimd
            # ring is software DGE executed on the Q7 cores -- never use it
            # for bulk). RoPE consts next; then wk/wv/wo.
            # X ships in batch-halves: wave Q(c=0) needs only xt[:, k, 0:T],
            # so its per-k working set is 384KB -- below the ring delivery
            # rate, which keeps the PE streaming (and HAM-warm) from the
            # first chunk
            for k in range(KT_HID):
                nc.sync.dma_start(out=xt_sb[:, k, 0:T], in_=hid_r[:, k, 0:T])
                nc.scalar.dma_start(out=wq_sb[:, k, :], in_=wq_r[:, k, :])
            for k in range(KT_HID):
                nc.sync.dma_start(out=xt_sb[:, k, T:TOK],
                                  in_=hid_r[:, k, T:TOK])
            nc.sync.dma_start(out=cosq_sb, in_=cos_q)
            nc.sync.dma_start(out=sinq_sb, in_=sin_q)
            nc.scalar.dma_start(out=cosk_sb, in_=cos_k)
            nc.scalar.dma_start(out=sink_sb, in_=sin_k)
            nc.scalar.dma_start(out=maskt_sb, in_=mask_t)
            for k in range(KT_HID):
                nc.sync.dma_start(out=wk_sb[:, k, :], in_=wk_r[:, k, :])
                nc.scalar.dma_start(out=wv_sb[:, k, :], in_=wv_r[:, k, :])

            # PE warmup: ~3.5us of dependency-light matmuls ahead of the
            # first projection so the HAM clock-gate releases (1.2 ->
            # 2.4 GHz) while the input DMAs are still in flight
            wps = psB.tile([P, T], FP32, tag="projps")
            for _ in range(16):
                nc.tensor.matmul(wps[:, 0:256], ones_bf, warm_rhs,
                                 start=True, stop=True, skip_group_check=True)

            def q_wave(c):
                # k-outer: the first matmuls need only (xt[0], wq[0]), so
                # the PE starts when the first 512KB lands, not after the
                # whole load; DMA delivery and PE consumption are balanced.
                pss = [psB.tile([P, T], FP32, tag="projps",
                                name=f"psq{c}_{i}") for i in range(NH)]
                for k in range(KT_HID):
                    for h in range(NH):
                        nc.tensor.matmul(
                            pss[h],
                            wq_sb[:, k, h * P:(h + 1) * P],
                            xt_sb[:, k, c * T:(c + 1) * T],
                            start=(k == 0), stop=(k == KT_HID - 1))
                for h in range(NH):
                    _rope(nc, qt_sb[:, h, c * T:(c + 1) * T], pss[h],
                          cosq_sb, sinq_sb, ropet)

            def k_wave():
                chains = [(g, cc) for g in range(NKV) for cc in range(BL)]
                pss = [psB.tile([P, T], FP32, tag="projps", name=f"psk{i}")
                       for i in range(len(chains))]
                for k in range(KT_HID):
                    for i, (g, cc) in enumerate(chains):
                        nc.tensor.matmul(
                            pss[i],
                            wk_sb[:, k, g * P:(g + 1) * P],
                            xt_sb[:, k, cc * T:(cc + 1) * T],
                            start=(k == 0), stop=(k == KT_HID - 1))
                for i, (g, cc) in enumerate(chains):
                    _rope(nc, kt_sb[:, g, cc * T:(cc + 1) * T], pss[i],
                          cosk_sb, sink_sb, ropet)

            def v_wave():
                pss = [psB.tile([P, T], FP32, tag="projps", name=f"psv{i}")
                       for i in range(NTOK_T)]
                for k in range(KT_HID):
                    for tt in range(NTOK_T):
                        nc.tensor.matmul(
                            pss[tt][:, :NKV * D],
                            xt_sb[:, k, tt * P:(tt + 1) * P],
                            wv_sb[:, k, :],
                            start=(k == 0), stop=(k == KT_HID - 1))
                for tt in range(NTOK_T):
                    # alternate copy engines so the drain is not ACT-serial
                    if tt % 2 == 0:
                        nc.scalar.copy(v_sb[:, tt, :], pss[tt][:, :NKV * D])
                    else:
                        nc.vector.tensor_copy(v_sb[:, tt, :],
                                              pss[tt][:, :NKV * D])

            # V last: the attention phase's first psum allocations alias
            # the last wave's banks, and V's copies drain fastest (a RoPE
            # tail would stall the first ST matmuls ~7us)
            q_wave(0)
            q_wave(1)
            k_wave()
            v_wave()

        # ---- phase C: attention, head pairs, one-pair software pipeline ----
        otpool = ctx.enter_context(tc.tile_pool(name="otpool", bufs=1))
        ot_sb = otpool.tile([P, NH, TOK], BF16, tag="ot")      # [d, h, tok]
        wopool = ctx.enter_context(tc.tile_pool(name="wopool", bufs=1))
        wo_sb = wopool.tile([P, KT_HID, HID], BF16, tag="wo")
        nc.sync.dma_start(out=wo_sb, in_=wo.rearrange("(k p) n -> p k n", p=P))
        # normalization state lives past phase C (batch 1's normalization
        # overlaps batch 0's output projection)
        normp = ctx.enter_context(tc.tile_pool(name="normp", bufs=8))
        sums = [normp.tile([NH, T], FP32, tag=f"sums{b}", name=f"sums{b}")
                for b in range(BL)]
        psR = ctx.enter_context(
            tc.tile_pool(name="psR", bufs=2, space=bass.MemorySpace.PSUM))

        with ExitStack() as phase2:
            ptpool = phase2.enter_context(tc.tile_pool(name="ptpool", bufs=2))
            # PSUM budget is exactly 8 banks: ST pair tiles 2x2 (per-j
            # tiles, double-buffered, so ST(j+1) never waits for exp(j) to
            # drain an overlapping region) + o_ps 1 + cs 1 (the cs/OT/copy
            # interleave within a pair covers the single-buffer WARs) +
            # rank-1 broadcast 2 (outer pool).
            psS = phase2.enter_context(
                tc.tile_pool(name="psS", bufs=2, space=bass.MemorySpace.PSUM))
            psO = phase2.enter_context(
                tc.tile_pool(name="psO", bufs=1, space=bass.MemorySpace.PSUM))
            psC = phase2.enter_context(
                tc.tile_pool(name="psC", bufs=1, space=bass.MemorySpace.PSUM))

            mask_b = maskt_sb[:, None, :].to_broadcast((P, GROUPS, P))

            def emit_st(b, g):
                """ST matmuls + exp + mask for one head pair; returns pt."""
                pt = ptpool.tile([P, GROUPS, NTQ, T], BF16, tag="pt")
                for j in range(NTQ):
                    lo = j * P
                    st = psS.tile([P, GROUPS, T], FP32, tag="sps")
                    for hh in range(GROUPS):
                        h = GROUPS * g + hh
                        nc.tensor.matmul(
                            st[:, hh, lo:T],
                            kt_sb[:, g, b * T + lo: b * T + lo + P],
                            qt_sb[:, h, b * T + lo: (b + 1) * T],
                            start=True, stop=True)
                    # one exp per (pair, j): 3D AP spanning both psum banks
                    # (no row-max: logits are O(1) by construction)
                    nc.scalar.activation(
                        out=pt[:, :, j, lo:T], in_=st[:, :, lo:T],
                        func=mybir.ActivationFunctionType.Exp,
                        bias=0.0, scale=1.0)
                    # causal mask on the diagonal block, both heads in one
                    # op (mask operand broadcast along the head axis), on
                    # the otherwise-idle gpsimd
                    nc.gpsimd.tensor_mul(
                        pt[:, :, j, lo:lo + P], pt[:, :, j, lo:lo + P],
                        mask_b)
                return pt

            def emit_csot_mm(b, g, pt):
                """colsum + OT accumulation; denominator stashed for the
                batched normalization."""
                for hh in range(GROUPS):
                    h = GROUPS * g + hh
                    o_ps = psO.tile([P, T], FP32, tag="ops")
                    cs_ps = psC.tile([1, T], FP32, tag="cps")
                    # colsum matmuls first so the denominator stash launches
                    # before the OT matmuls run
                    for j in range(NTQ):
                        lo = j * P
                        nc.tensor.matmul(
                            cs_ps[:, lo:T] if j else cs_ps[:, :],
                            ones_bf[:, 0:1],
                            pt[:, hh, j, lo:T],
                            start=(j == 0), stop=(j == NTQ - 1),
                            skip_group_check=True)
                    # copy the [1,T] row out of PSUM (split across ACT/DVE)
                    # and DMA-stash it into partition h of sums[b] so the
                    # whole batch's reciprocal runs 8-lane-wide later
                    # h0's copy on DVE (first DVE op of the pair, runs
                    # early), h1's on ACT: the next pair's first cs matmul
                    # WARs on h1's copy through the single cs bank, and ACT
                    # reaches it sooner than the backlogged DVE
                    csrow = normp.tile([1, T], FP32, tag="csrow")
                    if hh == 0:
                        nc.vector.tensor_copy(csrow, cs_ps)
                    else:
                        nc.scalar.copy(csrow, cs_ps)
                    nc.sync.dma_start(out=sums[b][h:h + 1, :], in_=csrow)
                    for j in range(NTQ):
                        lo = j * P
                        nc.tensor.matmul(
                            o_ps[:, lo:T] if j else o_ps[:, :],
                            v_sb[:, b * NTQ + j, g * D:(g + 1) * D],
                            pt[:, hh, j, lo:T],
                            start=(j == 0), stop=(j == NTQ - 1),
                            skip_group_check=True)
                    # unnormalized OT out of PSUM immediately (frees banks;
                    # the normalization multiply lands later, in-place)
                    nc.vector.tensor_copy(
                        ot_sb[:, h, b * T:(b + 1) * T], o_ps)

            def emit_batch_recip(b):
                """one 8-lane-wide reciprocal for the whole batch, then
                extract each head's row back to partition 0 for the rank-1
                broadcast. Runs while the next batch (or the output
                projection) keeps the PE busy."""
                rinv_f = normp.tile([NH, T], FP32, tag="rinvf",
                                    name=f"rinvf{b}")
                nc.vector.reciprocal_approx_fast(out=rinv_f, in_=sums[b])
                rinv_bf = normp.tile([NH, T], BF16, tag="rinvbf",
                                     name=f"rinvbf{b}")
                nc.vector.tensor_copy(rinv_bf, rinv_f)
                rrows = []
                for h in range(NH):
                    rrow = normp.tile([1, T], BF16, tag="rrow",
                                      name=f"rrow{b}_{h}")
                    nc.sync.dma_start(out=rrow, in_=rinv_bf[h:h + 1, :])
                    rrows.append(rrow)
                return rrows

            def emit_norm_heads(b, rrows, heads):
                """rank-1 broadcast + in-place multiply for a few heads;
                chunks are interspersed between matmul blocks so the PE
                never waits on the (cheap but latent) reciprocal chain."""
                for h in heads:
                    rb_ps = psR.tile([P, T], FP32, tag="rbps")
                    nc.tensor.matmul(rb_ps, ones_bf[0:1, :], rrows[h],
                                     start=True, stop=True,
                                     skip_group_check=True)
                    nc.vector.tensor_mul(
                        ot_sb[:, h, b * T:(b + 1) * T],
                        ot_sb[:, h, b * T:(b + 1) * T], rb_ps)

            # software-pipelined schedule: csot lags ST by one pair; batch
            # 0's normalization hides under batch 1's attention; batch 1's
            # hides under batch 0's output projection (emitted in phase D)
            pts = {}
            pairs = [(b, g) for b in range(BL) for g in range(NKV)]
            for b, g in pairs:
                pts[(b, g)] = None
            pts[(0, 0)] = emit_st(0, 0)
            for i in range(1, 4):
                pts[(0, i)] = emit_st(0, i)
                emit_csot_mm(0, i - 1, pts[(0, i - 1)])
            pts[(1, 0)] = emit_st(1, 0)
            emit_csot_mm(0, 3, pts[(0, 3)])
            pts[(1, 1)] = emit_st(1, 1)
            emit_csot_mm(1, 0, pts[(1, 0)])
            rrows0 = emit_batch_recip(0)
            emit_norm_heads(0, rrows0, [0, 1])
            pts[(1, 2)] = emit_st(1, 2)
            emit_csot_mm(1, 1, pts[(1, 1)])
            emit_norm_heads(0, rrows0, [2, 3])
            pts[(1, 3)] = emit_st(1, 3)
            emit_csot_mm(1, 2, pts[(1, 2)])
            emit_norm_heads(0, rrows0, [4, 5])
            emit_csot_mm(1, 3, pts[(1, 3)])
            emit_norm_heads(0, rrows0, [6, 7])
            rrows1 = emit_batch_recip(1)

        # ---- phase D: output projection (b0 tiles overlap b1's norm) ----
        with ExitStack() as phase3:
            opool = phase3.enter_context(tc.tile_pool(name="opool", bufs=3))
            psD = phase3.enter_context(
                tc.tile_pool(name="psD", bufs=3, space=bass.MemorySpace.PSUM))
            NCH = HID // T  # 2 chunks of 512

            def emit_out_tile(tt):
                o_tile = opool.tile([P, HID], BF16, tag="o")
                # interleave both output chunks k-major: consecutive matmul
                # pairs share the stationary operand OT[:,k,tt-block]
                ps0 = psD.tile([P, T], FP32, tag="dps0")
                ps1 = psD.tile([P, T], FP32, tag="dps1")
                pss = [ps0, ps1]
                for k in range(KT_HID):
                    for cchunk in range(NCH):
                        nc.tensor.matmul(
                            pss[cchunk],
                            ot_sb[:, k, tt * P:(tt + 1) * P],
                            wo_sb[:, k, cchunk * T:(cchunk + 1) * T],
                            start=(k == 0), stop=(k == KT_HID - 1))
                # alternate engines so the copies run in parallel; the last
                # tile is pure tail, so chunk it across engines and rings
                if tt < NTOK_T - 1:
                    nc.vector.tensor_copy(o_tile[:, 0:T], pss[0])
                    nc.scalar.copy(o_tile[:, T:HID], pss[1])
                    eng = nc.sync if tt % 2 == 0 else nc.scalar
                    eng.dma_start(out=out[tt * P:(tt + 1) * P, :], in_=o_tile)
                else:
                    HT = T // 2
                    nc.vector.tensor_copy(o_tile[:, 0:HT], pss[0][:, 0:HT])
                    nc.scalar.copy(o_tile[:, HT:T], pss[0][:, HT:T])
                    nc.vector.tensor_copy(o_tile[:, T:T + HT],
                                          pss[1][:, 0:HT])
                    nc.scalar.copy(o_tile[:, T + HT:HID], pss[1][:, HT:T])
                    nc.sync.dma_start(out=out[tt * P:(tt + 1) * P, 0:T],
                                      in_=o_tile[:, 0:T])
                    nc.scalar.dma_start(out=out[tt * P:(tt + 1) * P, T:HID],
                                        in_=o_tile[:, T:HID])

            emit_out_tile(0)
            emit_norm_heads(1, rrows1, [0, 1])
            emit_out_tile(1)
            emit_norm_heads(1, rrows1, [2, 3])
            emit_out_tile(2)
            emit_norm_heads(1, rrows1, [4, 5])
            emit_out_tile(3)
            emit_norm_heads(1, rrows1, [6, 7])
            for tt in range(4, NTOK_T):
                emit_out_tile(tt)


_COMPILED = None


def _get_compiled():
    global _COMPILED
    if _COMPILED is None:
        nc = bacc.Bacc("TRN2", target_bir_lowering=False, debug=False)
        _build(nc)
        nc.compile()
        _COMPILED = nc
    return _COMPILED


def kernel(hidden_states, Wq, Wk, Wv, Wo, _trace=False, _trace_kwargs=None):
    hs = np.asarray(hidden_states, dtype=np.float32).astype(BF)
    wq = np.ascontiguousarray(np.asarray(Wq, dtype=np.float32).astype(BF))
    wk = np.ascontiguousarray(np.asarray(Wk, dtype=np.float32).astype(BF))
    wv = np.ascontiguousarray(np.asarray(Wv, dtype=np.float32).astype(BF))
    wo = np.ascontiguousarray(np.asarray(Wo, dtype=np.float32).astype(BF))
    consts = _host_consts()
    nc = _get_compiled()
    in_maps = []
    for c in range(NCORES):
        # ship X pre-transposed ([HID, TOK]) so the kernel's lhs/rhs layouts
        # need no on-chip transpose of X at all
        shard_t = np.ascontiguousarray(
            hs[BL * c: BL * (c + 1)].reshape(TOK, HID).T)
        in_maps.append({"hidden_t": shard_t, "Wq": wq, "Wk": wk, "Wv": wv,
                        "Wo": wo, **consts})
    res = run_bass_kernel_spmd(
        nc, in_maps, list(range(NCORES)), trace=_trace,
        **(_trace_kwargs or {}))
    outs = [r["out"].astype(np.float32).reshape(BL, T, HID)
            for r in res.results]
    full = np.concatenate(outs, axis=0)
    if _trace:
        return full, res
    return full
